# revision 2
# baseline (speedup 1.0000x reference)
"""DeepAR (2-layer LSTM encoder + LSTM-cell decoder) Trainium2 Bass kernel.

Sharding: pure data parallel, batch 1024 -> 128 per core across 8 cores
(batch 128 == SBUF partition width).

Per-core design (v2 — fp8 DoubleRow encoder):
  - gates in [128 batch, 2048 gate] layout, gate order reordered to
    [g, i, f, o]: tanh(g) finishes first so the DVE chain starts early,
    and one sigmoid covers cols 512:2048.
  - encoder recurrent matmuls (h@W_hh0, h0@W_ih1, h1@W_hh1) run in
    fp8e4 with perf_mode=DoubleRow: 2 fp8 weights per PE cell -> ~2x
    matmul throughput. Weights and h both pre-scaled by 16 (so fp8
    mantissa sees normal-range values); the x-side weights/biases are
    scaled by 256 in bf16, and the cell activations apply scale=1/256.
  - h produced in bf16, transposed to stationary [K,M] layout with a
    SINGLE [128,512]->[128,4,128] xbar transpose per h (chunk-major
    layout verified on HW), then cast+scaled to fp8 on the idle Pool
    engine (keeps DVE/ACT FIFOs clean).
  - per-512-col n-chunk accumulation groups emitted so each PSUM bank
    completes early and ACT starts before the whole gate tensor is done.
  - layer 1 runs one step behind layer 0 so the PE always has
    independent matmul work while layer 0's elementwise chain runs.
  - decoder kept in bf16 (24 steps, accuracy headroom): context
    contribution precomputed once and injected into PSUM via identity
    matmul each step; mu/sigma heads are DVE dot-products.
"""
import numpy as np
import ml_dtypes

import concourse.bass as bass
import concourse.mybir as mybir
import concourse.tile as tile
from concourse import bacc
from concourse.bass_utils import run_bass_kernel_spmd
from concourse.masks import make_identity

F32 = mybir.dt.float32
BF16 = mybir.dt.bfloat16
FP8 = mybir.dt.float8e4
AF = mybir.ActivationFunctionType
ALU = mybir.AluOpType
DR = mybir.MatmulPerfMode.DoubleRow

B, T_ENC, H_DEC = 1024, 168, 24
ENC_IN, DEC_IN, HID = 32, 16, 512
G = 4 * HID  # 2048
NCORES = 8
BL = B // NCORES  # 128 batch per core
XCHUNK = 28  # encoder-input steps per DMA chunk

WSCALE = 16.0  # fp8 weight pre-scale
HSCALE = 16.0  # fp8 h pre-scale
GSCALE = 1.0 / (WSCALE * HSCALE)  # ACT de-scale on gate reads

# gate reorder: torch order [i, f, g, o] -> [g, i, f, o]
_PERM = np.concatenate([np.arange(1024, 1536), np.arange(0, 512),
                        np.arange(512, 1024), np.arange(1536, 2048)])


def _bf16(x):
    return np.ascontiguousarray(x.astype(ml_dtypes.bfloat16))


def _fp8(x):
    return np.ascontiguousarray(
        np.clip(x, -224.0, 224.0).astype(ml_dtypes.float8_e4m3))


def _f32(x):
    return np.ascontiguousarray(x.astype(np.float32))


def _wT_kxn(W, conv=_bf16, scale=1.0):
    """[4H, D] gate-major weight -> reordered W.T as [128, D//128, 4H]."""
    Wt = W[_PERM].T * scale  # [D, 2048]
    D = Wt.shape[0]
    return conv(Wt.reshape(D // 128, 128, G).transpose(1, 0, 2))


def build_kernel(T=T_ENC, HD=H_DEC):
    nc = bacc.Bacc("TRN2", target_bir_lowering=False, debug=False,
                   num_devices=NCORES)

    def din(name, shape, dt):
        return nc.dram_tensor(name, shape, dt, kind="ExternalInput").ap()

    x_d = din("x", [ENC_IN + 1, T, BL], BF16)        # enc features + ones row
    w0_d = din("w0", [ENC_IN + 1, G], BF16)           # (W_ih0T + bias row)*256
    wh0_d = din("wh0", [128, 4, G], FP8)              # *16
    wi1_d = din("wi1", [128, 4, G], FP8)              # *16
    wh1_d = din("wh1", [128, 4, G], FP8)              # *16
    wctx_d = din("wctx", [128, 4, G], BF16)
    whd_d = din("whd", [128, 4, G], BF16)
    be_d = din("be", [33, G + 128], BF16)  # row0: b1*256|ones, row32: bd|ones
    covy_d = din("covy", [DEC_IN + 1, HD, BL], BF16)  # dec covariates + y_prev
    wcy_d = din("wcy", [DEC_IN + 1, G], BF16)
    # head weights broadcast across partitions + per-partition biases:
    # cols 0:512 W_mu, 512:1024 W_sig, 1024 b_mu, 1025 b_sig
    wms_d = din("wms", [128, 2 * HID + 2], F32)

    mu_d = nc.dram_tensor("mu", [BL, HD], F32, kind="ExternalOutput").ap()
    sg_d = nc.dram_tensor("sg", [BL, HD], F32, kind="ExternalOutput").ap()

    with tile.TileContext(nc) as tc:
        _emit(tc, T, HD, x_d, w0_d, wh0_d, wi1_d, wh1_d, wctx_d, whd_d,
              be_d, covy_d, wcy_d, wms_d, mu_d, sg_d)
    nc.compile()
    return nc


def _emit(tc, T, HD, x_d, w0_d, wh0_d, wi1_d, wh1_d, wctx_d, whd_d,
          be_d, covy_d, wcy_d, wms_d, mu_d, sg_d):
    nc = tc.nc
    mm = nc.tensor.matmul

    with (
        tc.tile_pool(name="const", bufs=1) as cp,
        tc.tile_pool(name="xp", bufs=2) as xp,
        tc.tile_pool(name="sig", bufs=3) as sigp,
        tc.tile_pool(name="small", bufs=3) as smp,
        tc.tile_pool(name="hp", bufs=2) as hp,
        tc.tile_pool(name="htp", bufs=3) as htp,
        tc.tile_pool(name="ht8p", bufs=3) as ht8p,
        tc.tile_pool(name="psum", bufs=2, space="PSUM") as pp,
    ):
        # ---- persistent tiles / weight loads ----
        def load(name, dram, shape, dt):
            t = cp.tile(shape, dt, tag=name)
            nc.sync.dma_start(t[:], dram[:])
            return t

        w0 = load("w0", w0_d, [ENC_IN + 1, G], BF16)
        wh0 = load("wh0", wh0_d, [128, 4, G], FP8)
        be = load("be", be_d, [33, G + 128], BF16)
        wi1 = load("wi1", wi1_d, [128, 4, G], FP8)
        wh1 = load("wh1", wh1_d, [128, 4, G], FP8)

        ident = cp.tile([128, 128], BF16, tag="ident")
        make_identity(nc, ident[:])

        ones_r = be[0:1, G:G + 128]
        ones32_r = be[32:33, G:G + 128]
        b1_r = be[0:1, 0:G]
        bd_r = be[32:33, 0:G]

        c0 = cp.tile([128, HID], F32, tag="c0")
        c1 = cp.tile([128, HID], F32, tag="c1")
        cd = cp.tile([128, HID], F32, tag="cd")
        mu_b = cp.tile([128, HD], F32, tag="mu_b")
        sp_b = cp.tile([128, HD], F32, tag="sp_b")
        sg_b = cp.tile([128, HD], F32, tag="sg_b")

        NS = G // 512  # 4 n-chunks

        def cell(g, c, first, h_tag, scale=1.0):
            """gates psum [g|i|f|o] -> h (bf16 [128, HID]) via ACT/DVE.

            ACT order: tanh(g), sigmoid(i) first so DVE starts early.
            """
            tg = smp.tile([128, HID], F32, tag="tg")
            nc.scalar.activation(tg[:], g[:, 0:HID], AF.Tanh, scale=scale)
            si = smp.tile([128, HID], F32, tag="si")
            nc.scalar.activation(si[:], g[:, HID:2 * HID], AF.Sigmoid,
                                 scale=scale)
            sfo = sigp.tile([128, 2 * HID], F32, tag="sfo")
            nc.scalar.activation(sfo[:], g[:, 2 * HID:G], AF.Sigmoid,
                                 scale=scale)
            if first:
                nc.vector.tensor_mul(c[:], si[:], tg[:])
            else:
                m1 = smp.tile([128, HID], F32, tag="m1")
                nc.vector.tensor_mul(m1[:], si[:], tg[:])
                m2 = smp.tile([128, HID], F32, tag="m2")
                nc.vector.tensor_mul(m2[:], sfo[:, 0:HID], c[:])
                nc.vector.tensor_add(c[:], m1[:], m2[:])
            tcn = smp.tile([128, HID], F32, tag="tc")
            nc.scalar.activation(tcn[:], c[:], AF.Tanh)
            h = hp.tile([128, HID], BF16, tag=h_tag)
            hh = HID // 2
            nc.vector.tensor_mul(h[:, 0:hh], sfo[:, HID:HID + hh], tcn[:, 0:hh])
            nc.vector.tensor_mul(h[:, hh:HID], sfo[:, HID + hh:2 * HID],
                                 tcn[:, hh:HID])
            return h

        def transp(h, tag):
            """single-instruction [128,512] -> [128,4,128] xbar transpose."""
            ht = htp.tile([128, 4, 128], BF16, tag=tag)
            nc.sync.dma_start_transpose(ht[:], h[:])
            return ht

        def cast8(ht, tag):
            """bf16 hT -> fp8 (x16) on the idle Pool engine."""
            ht8 = ht8p.tile([128, 4, 128], FP8, tag=tag)
            nc.gpsimd.tensor_scalar_mul(ht8[:], ht[:], HSCALE)
            return ht8

        # ================= encoder =================
        # L1 runs one step behind L0: while L0(t)'s elementwise chain runs
        # on ACT/DVE/DMA, the PE stays busy on L1(t-1)'s matmuls.
        h0T8_hist = {}
        h1T8 = None
        h1T = None

        x_cur = None
        x_nxt = None

        def load_xchunk(t0):
            nxc = min(XCHUNK, T - t0)
            xt = xp.tile([ENC_IN + 1, XCHUNK, BL], BF16, tag="x")
            nc.sync.dma_start(xt[:, :nxc, :], x_d[:, t0:t0 + nxc, :])
            return xt

        for t in range(T):
            if t == 0:
                x_cur = load_xchunk(0)
                if T > XCHUNK:
                    x_nxt = load_xchunk(XCHUNK)
            elif t % XCHUNK == 0:
                x_cur = x_nxt
                if t + XCHUNK < T:
                    x_nxt = load_xchunk(t + XCHUNK)
            ti = t % XCHUNK

            # ---- layer 0 step t: input mms open the 4 psum groups ----
            g0 = pp.tile([128, G], F32, tag="g")
            for n in range(NS):
                s = slice(n * 512, (n + 1) * 512)
                mm(g0[:, s], x_cur[:, ti, :], w0[:, s],
                   start=True, stop=(t == 0))
            # L1(t-1) bias mms: always-ready PE filler while L0's recurrent
            # matmuls wait for the h0T8 transpose+cast.
            g1 = None
            if t >= 1:
                g1 = pp.tile([128, G], F32, tag="g")
                for n in range(NS):
                    s = slice(n * 512, (n + 1) * 512)
                    mm(g1[:, s], ones_r, b1_r[:, s], start=True, stop=False)
            # L0 recurrent, fp8 DoubleRow, n-outer so chunk n completes early
            if t > 0:
                hp8 = h0T8_hist[t - 1]
                for n in range(NS):
                    s = slice(n * 512, (n + 1) * 512)
                    mm(g0[:, s], hp8[:, 0:2, :], wh0[:, 0:2, s],
                       perf_mode=DR, start=False, stop=False)
                    mm(g0[:, s], hp8[:, 2:4, :], wh0[:, 2:4, s],
                       perf_mode=DR, start=False, stop=True)
            h0 = cell(g0, c0, t == 0, "h0", scale=GSCALE)
            h0T = transp(h0, "h0T")
            h0T8_hist[t] = cast8(h0T, "h0T8")
            h0T8_hist.pop(t - 2, None)

            # ---- layer 1, step t-1 ----
            if t >= 1:
                tp = t - 1
                hp8 = h0T8_hist[tp]
                for n in range(NS):
                    s = slice(n * 512, (n + 1) * 512)
                    mm(g1[:, s], hp8[:, 0:2, :], wi1[:, 0:2, s],
                       perf_mode=DR, start=False, stop=False)
                    mm(g1[:, s], hp8[:, 2:4, :], wi1[:, 2:4, s],
                       perf_mode=DR, start=False, stop=(tp == 0))
                    if tp > 0:
                        mm(g1[:, s], h1T8[:, 0:2, :], wh1[:, 0:2, s],
                           perf_mode=DR, start=False, stop=False)
                        mm(g1[:, s], h1T8[:, 2:4, :], wh1[:, 2:4, s],
                           perf_mode=DR, start=False, stop=True)
                h1 = cell(g1, c1, tp == 0, "h1", scale=GSCALE)
                h1T = transp(h1, "h1T")
                h1T8 = cast8(h1T, "h1T8")

        # final L1 step (t = T-1)
        g1 = pp.tile([128, G], F32, tag="g")
        for n in range(NS):
            s = slice(n * 512, (n + 1) * 512)
            mm(g1[:, s], ones_r, b1_r[:, s], start=True, stop=False)
        hp8 = h0T8_hist[T - 1]
        for n in range(NS):
            s = slice(n * 512, (n + 1) * 512)
            mm(g1[:, s], hp8[:, 0:2, :], wi1[:, 0:2, s],
               perf_mode=DR, start=False, stop=False)
            mm(g1[:, s], hp8[:, 2:4, :], wi1[:, 2:4, s],
               perf_mode=DR, start=False, stop=False)
            mm(g1[:, s], h1T8[:, 0:2, :], wh1[:, 0:2, s],
               perf_mode=DR, start=False, stop=False)
            mm(g1[:, s], h1T8[:, 2:4, :], wh1[:, 2:4, s],
               perf_mode=DR, start=False, stop=True)
        h1 = cell(g1, c1, False, "h1", scale=GSCALE)
        h1T = transp(h1, "h1T")

        # ================= decoder (bf16) =================
        wctx = load("wctx", wctx_d, [128, 4, G], BF16)
        whd = load("whd", whd_d, [128, 4, G], BF16)
        covy = load("covy", covy_d, [DEC_IN + 1, HD, BL], BF16)
        wcy = load("wcy", wcy_d, [DEC_IN + 1, G], BF16)
        wms = load("wms", wms_d, [128, 2 * HID + 2], F32)
        # one-time: ctx_pre = context @ W_ctx.T + (b_ihd + b_hhd)
        cps = pp.tile([128, G], F32, tag="g")
        for n in range(NS):
            s = slice(n * 512, (n + 1) * 512)
            mm(cps[:, s], ones32_r, bd_r[:, s], start=True, stop=False)
        for k in range(4):
            for n in range(NS):
                s = slice(n * 512, (n + 1) * 512)
                mm(cps[:, s], h1T[:, k, :], wctx[:, k, s],
                   start=False, stop=(k == 3))
        ctxp = cp.tile([128, G], BF16, tag="ctxp")
        nc.scalar.copy(ctxp[:], cps[:])

        hdT = None
        for t in range(HD):
            gd = pp.tile([128, G], F32, tag="g")
            for n in range(NS):
                s = slice(n * 512, (n + 1) * 512)
                mm(gd[:, s], ident[:], ctxp[:, s], start=True, stop=False)
                mm(gd[:, s], covy[:, t, :], wcy[:, s],
                   start=False, stop=(t == 0))
            if t > 0:
                for k in range(4):
                    for n in range(NS):
                        s = slice(n * 512, (n + 1) * 512)
                        mm(gd[:, s], hdT[:, k, :], whd[:, k, s],
                           start=False, stop=(k == 3))
            hd = cell(gd, cd, t == 0, "hd")
            hdT = transp(hd, "hdT")

            # heads: mu/sigma dot-products on DVE, off the critical path
            hsc = smp.tile([128, HID], F32, tag="hsc")
            nc.vector.scalar_tensor_tensor(
                hsc[:], hd[:], 1.0, wms[:, 0:HID],
                op0=ALU.mult, op1=ALU.mult, accum_out=mu_b[:, t:t + 1])
            hsc2 = smp.tile([128, HID], F32, tag="hsc2")
            nc.vector.scalar_tensor_tensor(
                hsc2[:], hd[:], 1.0, wms[:, HID:2 * HID],
                op0=ALU.mult, op1=ALU.mult, accum_out=sp_b[:, t:t + 1])

        # add head biases; sigma = softplus(x) + 1e-6 via ln(exp(x)+1)
        nc.vector.tensor_scalar_add(mu_b[:], mu_b[:],
                                    wms[:, 2 * HID:2 * HID + 1])
        nc.vector.tensor_scalar_add(sp_b[:], sp_b[:],
                                    wms[:, 2 * HID + 1:2 * HID + 2])
        nc.scalar.activation(sp_b[:], sp_b[:], AF.Exp)
        nc.scalar.activation(sg_b[:], sp_b[:], AF.Ln, bias=1.0)
        nc.vector.tensor_scalar_add(sg_b[:], sg_b[:], 1e-6)
        nc.sync.dma_start(mu_d[:], mu_b[:])
        nc.sync.dma_start(sg_d[:], sg_b[:])


def _make_be(b1, bdv):
    be = np.zeros((33, G + 128), np.float32)
    be[0, :G] = b1 / GSCALE  # *256: de-scaled by the ACT gate reads
    be[32, :G] = bdv
    be[0, G:] = 1.0
    be[32, G:] = 1.0
    return _bf16(be)


def _make_wms(W_mu, W_sig, b_mu, b_sig):
    w = np.zeros((128, 2 * HID + 2), np.float32)
    w[:, 0:HID] = W_mu[0][None, :]
    w[:, HID:2 * HID] = W_sig[0][None, :]
    w[:, 2 * HID] = b_mu[0]
    w[:, 2 * HID + 1] = b_sig[0]
    return _f32(w)


def prep_inputs(inputs, T=T_ENC, HD=H_DEC):
    """Full-batch inputs -> list of per-core input maps (host layout prep)."""
    enc = _f32(np.asarray(inputs["enc_inp"]))[:, :T]
    dec = _f32(np.asarray(inputs["dec_inp"]))[:, :HD]
    tgt = _f32(np.asarray(inputs["tgt"]))[:, :HD]

    W_ih0, W_hh0 = np.asarray(inputs["W_ih0"]), np.asarray(inputs["W_hh0"])
    W_ih1, W_hh1 = np.asarray(inputs["W_ih1"]), np.asarray(inputs["W_hh1"])
    W_ihd, W_hhd = np.asarray(inputs["W_ihd"]), np.asarray(inputs["W_hhd"])
    b0 = _f32(np.asarray(inputs["b_ih0"]) + np.asarray(inputs["b_hh0"]))[_PERM]
    b1 = _f32(np.asarray(inputs["b_ih1"]) + np.asarray(inputs["b_hh1"]))[_PERM]
    bdv = _f32(np.asarray(inputs["b_ihd"]) + np.asarray(inputs["b_hhd"]))[_PERM]
    W_mu, b_mu = np.asarray(inputs["W_mu"]), np.asarray(inputs["b_mu"])
    W_sig, b_sig = np.asarray(inputs["W_sig"]), np.asarray(inputs["b_sig"])

    # x-side weights *256 in bf16 (exact power-of-two scale); gate reads
    # apply scale=1/256. b0 rides the ones-row of x.
    w0 = np.concatenate([W_ih0[_PERM].T, b0[None, :]], 0) / GSCALE  # [33,2048]
    shared = {
        "w0": _bf16(w0),
        "wh0": _wT_kxn(W_hh0, conv=_fp8, scale=WSCALE),
        "wi1": _wT_kxn(W_ih1, conv=_fp8, scale=WSCALE),
        "wh1": _wT_kxn(W_hh1, conv=_fp8, scale=WSCALE),
        "wctx": _wT_kxn(W_ihd[:, DEC_IN:DEC_IN + HID]),
        "whd": _wT_kxn(W_hhd),
        "be": _make_be(b1, bdv),
        "wcy": _bf16(np.concatenate(
            [W_ihd[_PERM][:, :DEC_IN].T, W_ihd[_PERM][:, DEC_IN + HID:].T], 0)),
        "wms": _make_wms(W_mu, W_sig, b_mu, b_sig),
    }

    in_maps = []
    for c in range(NCORES):
        sl = slice(c * BL, (c + 1) * BL)
        xe = np.ones((ENC_IN + 1, T, BL), np.float32)
        xe[:ENC_IN] = enc[sl].transpose(2, 1, 0)
        cy = np.zeros((DEC_IN + 1, HD, BL), np.float32)
        cy[:DEC_IN] = dec[sl].transpose(2, 1, 0)
        cy[DEC_IN, 1:] = tgt[sl, :HD - 1].T
        m = dict(shared)
        m["x"] = _bf16(xe)
        m["covy"] = _bf16(cy)
        in_maps.append(m)
    return in_maps


_NC_CACHE = {}


def _get_nc(T=T_ENC, HD=H_DEC):
    key = (T, HD)
    if key not in _NC_CACHE:
        _NC_CACHE[key] = build_kernel(T, HD)
    return _NC_CACHE[key]


def run(inputs, T=T_ENC, HD=H_DEC, **kw):
    nc = _get_nc(T, HD)
    in_maps = prep_inputs(inputs, T, HD)
    res = run_bass_kernel_spmd(nc, in_maps, core_ids=list(range(NCORES)), **kw)
    mu = np.concatenate([res.results[c]["mu"] for c in range(NCORES)], 0)
    sg = np.concatenate([res.results[c]["sg"] for c in range(NCORES)], 0)
    return (mu, sg), res


def kernel(**inputs):
    (mu, sg), _ = run(inputs)
    return mu, sg


# revision 7
# speedup vs baseline: 1.2025x; 1.2025x over previous
"""DeepAR (2-layer LSTM encoder + LSTM-cell decoder) Trainium2 Bass kernel.

Sharding: pure data parallel, batch 1024 -> 128 per core across 8 cores
(batch 128 == SBUF partition width).

Per-core design (v2 — fp8 DoubleRow encoder):
  - gates in [128 batch, 2048 gate] layout, gate order reordered to
    [g, i, f, o]: tanh(g) finishes first so the DVE chain starts early,
    and one sigmoid covers cols 512:2048.
  - encoder recurrent matmuls (h@W_hh0, h0@W_ih1, h1@W_hh1) run in
    fp8e4 with perf_mode=DoubleRow: 2 fp8 weights per PE cell -> ~2x
    matmul throughput. Weights and h both pre-scaled by 16 (so fp8
    mantissa sees normal-range values); the x-side weights/biases are
    scaled by 256 in bf16, and the cell activations apply scale=1/256.
  - h produced in bf16, transposed to stationary [K,M] layout with a
    SINGLE [128,512]->[128,4,128] xbar transpose per h (chunk-major
    layout verified on HW), then cast+scaled to fp8 on the idle Pool
    engine (keeps DVE/ACT FIFOs clean).
  - per-512-col n-chunk accumulation groups emitted so each PSUM bank
    completes early and ACT starts before the whole gate tensor is done.
  - layer 1 runs one step behind layer 0 so the PE always has
    independent matmul work while layer 0's elementwise chain runs.
  - decoder kept in bf16 (24 steps, accuracy headroom): context
    contribution precomputed once and injected into PSUM via identity
    matmul each step; mu/sigma heads are DVE dot-products.
"""
import numpy as np
import ml_dtypes

import concourse.bass as bass
import concourse.mybir as mybir
import concourse.tile as tile
from concourse import bacc
from concourse.bass_utils import run_bass_kernel_spmd
from concourse.masks import make_identity

F32 = mybir.dt.float32
BF16 = mybir.dt.bfloat16
FP8 = mybir.dt.float8e4
AF = mybir.ActivationFunctionType
ALU = mybir.AluOpType
DR = mybir.MatmulPerfMode.DoubleRow

B, T_ENC, H_DEC = 1024, 168, 24
ENC_IN, DEC_IN, HID = 32, 16, 512
G = 4 * HID  # 2048
NCORES = 8
BL = B // NCORES  # 128 batch per core
XCHUNK = 28  # encoder-input steps per DMA chunk

WSCALE = 16.0  # fp8 weight pre-scale
HSCALE = 16.0  # fp8 h pre-scale
GSCALE = 1.0 / (WSCALE * HSCALE)  # ACT de-scale on gate reads

# gate reorder: torch order [i, f, g, o] -> [g, i, f, o]
_PERM = np.concatenate([np.arange(1024, 1536), np.arange(0, 512),
                        np.arange(512, 1024), np.arange(1536, 2048)])


def _bf16(x):
    return np.ascontiguousarray(x.astype(ml_dtypes.bfloat16))


def _fp8(x):
    return np.ascontiguousarray(
        np.clip(x, -224.0, 224.0).astype(ml_dtypes.float8_e4m3))


def _f32(x):
    return np.ascontiguousarray(x.astype(np.float32))


def _wT_kxn(W, conv=_bf16, scale=1.0):
    """[4H, D] gate-major weight -> reordered W.T as [128, D//128, 4H]."""
    Wt = W[_PERM].T * scale  # [D, 2048]
    D = Wt.shape[0]
    return conv(Wt.reshape(D // 128, 128, G).transpose(1, 0, 2))


def build_kernel(T=T_ENC, HD=H_DEC):
    nc = bacc.Bacc("TRN2", target_bir_lowering=False, debug=False,
                   num_devices=NCORES)

    def din(name, shape, dt):
        return nc.dram_tensor(name, shape, dt, kind="ExternalInput").ap()

    x_d = din("x", [ENC_IN + 1, T, BL], BF16)        # enc features + ones row
    w0_d = din("w0", [ENC_IN + 1, G], BF16)           # (W_ih0T + bias row)*256
    wh0_d = din("wh0", [128, 4, G], FP8)              # *16
    wi1_d = din("wi1", [128, 4, G], FP8)              # *16
    wh1_d = din("wh1", [128, 4, G], FP8)              # *16
    wctx_d = din("wctx", [128, 4, G], BF16)
    whd_d = din("whd", [128, 4, G], BF16)
    be_d = din("be", [33, G + 128], BF16)  # row0: b1*256|ones, row32: bd|ones
    covy_d = din("covy", [DEC_IN + 1, HD, BL], BF16)  # dec covariates + y_prev
    wcy_d = din("wcy", [DEC_IN + 1, G], BF16)
    # head weights broadcast across partitions + per-partition biases:
    # cols 0:512 W_mu, 512:1024 W_sig, 1024 b_mu, 1025 b_sig
    wms_d = din("wms", [128, 2 * HID + 2], F32)

    mu_d = nc.dram_tensor("mu", [BL, HD], F32, kind="ExternalOutput").ap()
    sg_d = nc.dram_tensor("sg", [BL, HD], F32, kind="ExternalOutput").ap()

    with tile.TileContext(nc) as tc:
        _emit(tc, T, HD, x_d, w0_d, wh0_d, wi1_d, wh1_d, wctx_d, whd_d,
              be_d, covy_d, wcy_d, wms_d, mu_d, sg_d)
    nc.compile()
    return nc


def _emit(tc, T, HD, x_d, w0_d, wh0_d, wi1_d, wh1_d, wctx_d, whd_d,
          be_d, covy_d, wcy_d, wms_d, mu_d, sg_d):
    nc = tc.nc
    mm = nc.tensor.matmul

    with (
        tc.tile_pool(name="const", bufs=1) as cp,
        tc.tile_pool(name="xp", bufs=2) as xp,
        tc.tile_pool(name="sig", bufs=3) as sigp,
        tc.tile_pool(name="small", bufs=3) as smp,
        tc.tile_pool(name="hp", bufs=2) as hp,
        tc.tile_pool(name="htp", bufs=3) as htp,
        tc.tile_pool(name="ht8p", bufs=4) as ht8p,
        tc.tile_pool(name="psum", bufs=2, space="PSUM") as pp,
    ):
        # ---- persistent tiles / weight loads ----
        def load(name, dram, shape, dt):
            t = cp.tile(shape, dt, tag=name)
            nc.sync.dma_start(t[:], dram[:])
            return t

        w0 = load("w0", w0_d, [ENC_IN + 1, G], BF16)
        wh0 = load("wh0", wh0_d, [128, 4, G], FP8)
        be = load("be", be_d, [33, G + 128], BF16)
        wi1 = load("wi1", wi1_d, [128, 4, G], FP8)
        wh1 = load("wh1", wh1_d, [128, 4, G], FP8)

        ident = cp.tile([128, 128], BF16, tag="ident")
        make_identity(nc, ident[:])

        ones_r = be[0:1, G:G + 128]
        ones32_r = be[32:33, G:G + 128]
        b1_r = be[0:1, 0:G]
        bd_r = be[32:33, 0:G]

        c0 = cp.tile([128, HID], F32, tag="c0")
        c1 = cp.tile([128, HID], F32, tag="c1")
        cd = cp.tile([128, HID], F32, tag="cd")
        mu_b = cp.tile([128, HD], F32, tag="mu_b")
        sp_b = cp.tile([128, HD], F32, tag="sp_b")
        sg_b = cp.tile([128, HD], F32, tag="sg_b")

        NS = G // 512  # 4 n-chunks

        def cell(g, c, first, h_tag, scale=1.0):
            """gates psum [g|i|f|o] -> h (bf16 [128, HID]) via ACT/DVE.

            ACT order: tanh(g), sigmoid(i) first so DVE starts early.
            """
            tg = smp.tile([128, HID], F32, tag="tg")
            nc.scalar.activation(tg[:], g[:, 0:HID], AF.Tanh, scale=scale)
            si = smp.tile([128, HID], F32, tag="si")
            nc.scalar.activation(si[:], g[:, HID:2 * HID], AF.Sigmoid,
                                 scale=scale)
            sfo = sigp.tile([128, 2 * HID], F32, tag="sfo")
            nc.scalar.activation(sfo[:], g[:, 2 * HID:G], AF.Sigmoid,
                                 scale=scale)
            if first:
                nc.vector.tensor_mul(c[:], si[:], tg[:])
            else:
                m1 = smp.tile([128, HID], F32, tag="m1")
                nc.vector.tensor_mul(m1[:], si[:], tg[:])
                m2 = smp.tile([128, HID], F32, tag="m2")
                nc.vector.tensor_mul(m2[:], sfo[:, 0:HID], c[:])
                nc.vector.tensor_add(c[:], m1[:], m2[:])
            tcn = smp.tile([128, HID], F32, tag="tc")
            nc.scalar.activation(tcn[:], c[:], AF.Tanh)
            h = hp.tile([128, HID], BF16, tag=h_tag)
            hh = HID // 2
            nc.vector.tensor_mul(h[:, 0:hh], sfo[:, HID:HID + hh], tcn[:, 0:hh])
            nc.vector.tensor_mul(h[:, hh:HID], sfo[:, HID + hh:2 * HID],
                                 tcn[:, hh:HID])
            return h

        def transp(h, tag):
            """single-instruction [128,512] -> [128,4,128] xbar transpose."""
            ht = htp.tile([128, 4, 128], BF16, tag=tag)
            nc.sync.dma_start_transpose(ht[:], h[:])
            return ht

        def cast8(ht, tag):
            """bf16 hT -> fp8 (x16) on DVE. Emission point chosen so the
            strict DVE FIFO never idle-waits on the transpose DMA."""
            ht8 = ht8p.tile([128, 4, 128], FP8, tag=tag)
            nc.vector.tensor_scalar_mul(ht8[:], ht[:], HSCALE)
            return ht8

        # ================= encoder =================
        # L1 runs one step behind L0: while L0(t)'s elementwise chain runs
        # on ACT/DVE/DMA, the PE stays busy on L1(t-1)'s matmuls.
        h0T8_hist = {}
        h1T8 = None
        h1T = None
        h1T_pending = None  # bf16 transposed h1 awaiting its DVE fp8 cast

        x_cur = None
        x_nxt = None

        def load_xchunk(t0):
            nxc = min(XCHUNK, T - t0)
            xt = xp.tile([ENC_IN + 1, XCHUNK, BL], BF16, tag="x")
            nc.sync.dma_start(xt[:, :nxc, :], x_d[:, t0:t0 + nxc, :])
            return xt

        for t in range(T):
            if t == 0:
                x_cur = load_xchunk(0)
                if T > XCHUNK:
                    x_nxt = load_xchunk(XCHUNK)
            elif t % XCHUNK == 0:
                x_cur = x_nxt
                if t + XCHUNK < T:
                    x_nxt = load_xchunk(t + XCHUNK)
            ti = t % XCHUNK

            # ---- layer 0 step t: input mms open the 4 psum groups ----
            g0 = pp.tile([128, G], F32, tag="g")
            for n in range(NS):
                s = slice(n * 512, (n + 1) * 512)
                mm(g0[:, s], x_cur[:, ti, :], w0[:, s],
                   start=True, stop=(t == 0))
            # L1(t-1) bias mms: always-ready PE filler while L0's recurrent
            # matmuls wait for the h0T8 transpose+cast.
            g1 = None
            if t >= 1:
                g1 = pp.tile([128, G], F32, tag="g")
                for n in range(NS):
                    s = slice(n * 512, (n + 1) * 512)
                    mm(g1[:, s], ones_r, b1_r[:, s], start=True, stop=False)
            # first DVE op of the iteration: cast last iteration's h1T
            # (its transpose completed around the iteration boundary).
            if h1T_pending is not None:
                h1T8 = cast8(h1T_pending, "h1T8")
                h1T_pending = None
            # L0 recurrent, fp8 DoubleRow, n-outer so chunk n completes early
            if t > 0:
                hp8 = h0T8_hist[t - 1]
                for n in range(NS):
                    s = slice(n * 512, (n + 1) * 512)
                    mm(g0[:, s], hp8[:, 0:2, :], wh0[:, 0:2, s],
                       perf_mode=DR, start=False, stop=False)
                    mm(g0[:, s], hp8[:, 2:4, :], wh0[:, 2:4, s],
                       perf_mode=DR, start=False, stop=True)
            h0 = cell(g0, c0, t == 0, "h0", scale=GSCALE)
            h0T = transp(h0, "h0T")

            # ---- layer 1, step t-1 ----
            if t >= 1:
                tp = t - 1
                hp8 = h0T8_hist[tp]
                for n in range(NS):
                    s = slice(n * 512, (n + 1) * 512)
                    mm(g1[:, s], hp8[:, 0:2, :], wi1[:, 0:2, s],
                       perf_mode=DR, start=False, stop=False)
                    mm(g1[:, s], hp8[:, 2:4, :], wi1[:, 2:4, s],
                       perf_mode=DR, start=False, stop=(tp == 0))
                    if tp > 0:
                        mm(g1[:, s], h1T8[:, 0:2, :], wh1[:, 0:2, s],
                           perf_mode=DR, start=False, stop=False)
                        mm(g1[:, s], h1T8[:, 2:4, :], wh1[:, 2:4, s],
                           perf_mode=DR, start=False, stop=True)
                h1 = cell(g1, c1, tp == 0, "h1", scale=GSCALE)
                h1T = transp(h1, "h1T")
                h1T_pending = h1T
            # last DVE op of the iteration: cast this step's h0T (its
            # transpose completed during the L1 matmul/cell phase).
            h0T8_hist[t] = cast8(h0T, "h0T8")
            h0T8_hist.pop(t - 2, None)

        # final L1 step (t = T-1)
        g1 = pp.tile([128, G], F32, tag="g")
        for n in range(NS):
            s = slice(n * 512, (n + 1) * 512)
            mm(g1[:, s], ones_r, b1_r[:, s], start=True, stop=False)
        h1T8 = cast8(h1T_pending, "h1T8")
        hp8 = h0T8_hist[T - 1]
        for n in range(NS):
            s = slice(n * 512, (n + 1) * 512)
            mm(g1[:, s], hp8[:, 0:2, :], wi1[:, 0:2, s],
               perf_mode=DR, start=False, stop=False)
            mm(g1[:, s], hp8[:, 2:4, :], wi1[:, 2:4, s],
               perf_mode=DR, start=False, stop=False)
            mm(g1[:, s], h1T8[:, 0:2, :], wh1[:, 0:2, s],
               perf_mode=DR, start=False, stop=False)
            mm(g1[:, s], h1T8[:, 2:4, :], wh1[:, 2:4, s],
               perf_mode=DR, start=False, stop=True)
        h1 = cell(g1, c1, False, "h1", scale=GSCALE)
        h1T = transp(h1, "h1T")

        # ================= decoder (bf16) =================
        wctx = load("wctx", wctx_d, [128, 4, G], BF16)
        whd = load("whd", whd_d, [128, 4, G], BF16)
        covy = load("covy", covy_d, [DEC_IN + 1, HD, BL], BF16)
        wcy = load("wcy", wcy_d, [DEC_IN + 1, G], BF16)
        wms = load("wms", wms_d, [128, 2 * HID + 2], F32)
        # one-time: ctx_pre = context @ W_ctx.T + (b_ihd + b_hhd)
        cps = pp.tile([128, G], F32, tag="g")
        for n in range(NS):
            s = slice(n * 512, (n + 1) * 512)
            mm(cps[:, s], ones32_r, bd_r[:, s], start=True, stop=False)
        for k in range(4):
            for n in range(NS):
                s = slice(n * 512, (n + 1) * 512)
                mm(cps[:, s], h1T[:, k, :], wctx[:, k, s],
                   start=False, stop=(k == 3))
        ctxp = cp.tile([128, G], BF16, tag="ctxp")
        nc.scalar.copy(ctxp[:], cps[:])

        hdT = None
        for t in range(HD):
            gd = pp.tile([128, G], F32, tag="g")
            for n in range(NS):
                s = slice(n * 512, (n + 1) * 512)
                mm(gd[:, s], ident[:], ctxp[:, s], start=True, stop=False)
                mm(gd[:, s], covy[:, t, :], wcy[:, s],
                   start=False, stop=(t == 0))
            if t > 0:
                for k in range(4):
                    for n in range(NS):
                        s = slice(n * 512, (n + 1) * 512)
                        mm(gd[:, s], hdT[:, k, :], whd[:, k, s],
                           start=False, stop=(k == 3))
            hd = cell(gd, cd, t == 0, "hd")
            hdT = transp(hd, "hdT")

            # heads: mu/sigma dot-products on DVE, off the critical path
            hsc = smp.tile([128, HID], F32, tag="hsc")
            nc.vector.scalar_tensor_tensor(
                hsc[:], hd[:], 1.0, wms[:, 0:HID],
                op0=ALU.mult, op1=ALU.mult, accum_out=mu_b[:, t:t + 1])
            hsc2 = smp.tile([128, HID], F32, tag="hsc2")
            nc.vector.scalar_tensor_tensor(
                hsc2[:], hd[:], 1.0, wms[:, HID:2 * HID],
                op0=ALU.mult, op1=ALU.mult, accum_out=sp_b[:, t:t + 1])

        # add head biases; sigma = softplus(x) + 1e-6 via ln(exp(x)+1)
        nc.vector.tensor_scalar_add(mu_b[:], mu_b[:],
                                    wms[:, 2 * HID:2 * HID + 1])
        nc.vector.tensor_scalar_add(sp_b[:], sp_b[:],
                                    wms[:, 2 * HID + 1:2 * HID + 2])
        nc.scalar.activation(sp_b[:], sp_b[:], AF.Exp)
        nc.scalar.activation(sg_b[:], sp_b[:], AF.Ln, bias=1.0)
        nc.vector.tensor_scalar_add(sg_b[:], sg_b[:], 1e-6)
        nc.sync.dma_start(mu_d[:], mu_b[:])
        nc.sync.dma_start(sg_d[:], sg_b[:])


def _make_be(b1, bdv):
    be = np.zeros((33, G + 128), np.float32)
    be[0, :G] = b1 / GSCALE  # *256: de-scaled by the ACT gate reads
    be[32, :G] = bdv
    be[0, G:] = 1.0
    be[32, G:] = 1.0
    return _bf16(be)


def _make_wms(W_mu, W_sig, b_mu, b_sig):
    w = np.zeros((128, 2 * HID + 2), np.float32)
    w[:, 0:HID] = W_mu[0][None, :]
    w[:, HID:2 * HID] = W_sig[0][None, :]
    w[:, 2 * HID] = b_mu[0]
    w[:, 2 * HID + 1] = b_sig[0]
    return _f32(w)


def prep_inputs(inputs, T=T_ENC, HD=H_DEC):
    """Full-batch inputs -> list of per-core input maps (host layout prep)."""
    enc = _f32(np.asarray(inputs["enc_inp"]))[:, :T]
    dec = _f32(np.asarray(inputs["dec_inp"]))[:, :HD]
    tgt = _f32(np.asarray(inputs["tgt"]))[:, :HD]

    W_ih0, W_hh0 = np.asarray(inputs["W_ih0"]), np.asarray(inputs["W_hh0"])
    W_ih1, W_hh1 = np.asarray(inputs["W_ih1"]), np.asarray(inputs["W_hh1"])
    W_ihd, W_hhd = np.asarray(inputs["W_ihd"]), np.asarray(inputs["W_hhd"])
    b0 = _f32(np.asarray(inputs["b_ih0"]) + np.asarray(inputs["b_hh0"]))[_PERM]
    b1 = _f32(np.asarray(inputs["b_ih1"]) + np.asarray(inputs["b_hh1"]))[_PERM]
    bdv = _f32(np.asarray(inputs["b_ihd"]) + np.asarray(inputs["b_hhd"]))[_PERM]
    W_mu, b_mu = np.asarray(inputs["W_mu"]), np.asarray(inputs["b_mu"])
    W_sig, b_sig = np.asarray(inputs["W_sig"]), np.asarray(inputs["b_sig"])

    # x-side weights *256 in bf16 (exact power-of-two scale); gate reads
    # apply scale=1/256. b0 rides the ones-row of x.
    w0 = np.concatenate([W_ih0[_PERM].T, b0[None, :]], 0) / GSCALE  # [33,2048]
    shared = {
        "w0": _bf16(w0),
        "wh0": _wT_kxn(W_hh0, conv=_fp8, scale=WSCALE),
        "wi1": _wT_kxn(W_ih1, conv=_fp8, scale=WSCALE),
        "wh1": _wT_kxn(W_hh1, conv=_fp8, scale=WSCALE),
        "wctx": _wT_kxn(W_ihd[:, DEC_IN:DEC_IN + HID]),
        "whd": _wT_kxn(W_hhd),
        "be": _make_be(b1, bdv),
        "wcy": _bf16(np.concatenate(
            [W_ihd[_PERM][:, :DEC_IN].T, W_ihd[_PERM][:, DEC_IN + HID:].T], 0)),
        "wms": _make_wms(W_mu, W_sig, b_mu, b_sig),
    }

    in_maps = []
    for c in range(NCORES):
        sl = slice(c * BL, (c + 1) * BL)
        xe = np.ones((ENC_IN + 1, T, BL), np.float32)
        xe[:ENC_IN] = enc[sl].transpose(2, 1, 0)
        cy = np.zeros((DEC_IN + 1, HD, BL), np.float32)
        cy[:DEC_IN] = dec[sl].transpose(2, 1, 0)
        cy[DEC_IN, 1:] = tgt[sl, :HD - 1].T
        m = dict(shared)
        m["x"] = _bf16(xe)
        m["covy"] = _bf16(cy)
        in_maps.append(m)
    return in_maps


_NC_CACHE = {}


def _get_nc(T=T_ENC, HD=H_DEC):
    key = (T, HD)
    if key not in _NC_CACHE:
        _NC_CACHE[key] = build_kernel(T, HD)
    return _NC_CACHE[key]


def run(inputs, T=T_ENC, HD=H_DEC, **kw):
    nc = _get_nc(T, HD)
    in_maps = prep_inputs(inputs, T, HD)
    res = run_bass_kernel_spmd(nc, in_maps, core_ids=list(range(NCORES)), **kw)
    mu = np.concatenate([res.results[c]["mu"] for c in range(NCORES)], 0)
    sg = np.concatenate([res.results[c]["sg"] for c in range(NCORES)], 0)
    return (mu, sg), res


def kernel(**inputs):
    (mu, sg), _ = run(inputs)
    return mu, sg


# revision 10
# speedup vs baseline: 1.5314x; 1.2736x over previous
"""DeepAR (2-layer LSTM encoder + LSTM-cell decoder) Trainium2 Bass kernel.

Sharding: pure data parallel, batch 1024 -> 128 per core across 8 cores
(batch 128 == SBUF partition width).

Per-core design (v2 — fp8 DoubleRow encoder):
  - gates in [128 batch, 2048 gate] layout, gate order reordered to
    [g, i, f, o]: tanh(g) finishes first so the DVE chain starts early,
    and one sigmoid covers cols 512:2048.
  - encoder recurrent matmuls (h@W_hh0, h0@W_ih1, h1@W_hh1) run in
    fp8e4 with perf_mode=DoubleRow: 2 fp8 weights per PE cell -> ~2x
    matmul throughput. Weights and h both pre-scaled by 16 (so fp8
    mantissa sees normal-range values); the x-side weights/biases are
    scaled by 256 in bf16, and the cell activations apply scale=1/256.
  - h produced in bf16, transposed to stationary [K,M] layout with a
    SINGLE [128,512]->[128,4,128] xbar transpose per h (chunk-major
    layout verified on HW), then cast+scaled to fp8 on the idle Pool
    engine (keeps DVE/ACT FIFOs clean).
  - per-512-col n-chunk accumulation groups emitted so each PSUM bank
    completes early and ACT starts before the whole gate tensor is done.
  - layer 1 runs one step behind layer 0 so the PE always has
    independent matmul work while layer 0's elementwise chain runs.
  - decoder kept in bf16 (24 steps, accuracy headroom): context
    contribution precomputed once and injected into PSUM via identity
    matmul each step; mu/sigma heads are DVE dot-products.
"""
import numpy as np
import ml_dtypes

import concourse.bass as bass
import concourse.mybir as mybir
import concourse.tile as tile
from concourse import bacc
from concourse.bass_utils import run_bass_kernel_spmd
from concourse.masks import make_identity

F32 = mybir.dt.float32
BF16 = mybir.dt.bfloat16
FP8 = mybir.dt.float8e4
AF = mybir.ActivationFunctionType
ALU = mybir.AluOpType
DR = mybir.MatmulPerfMode.DoubleRow

B, T_ENC, H_DEC = 1024, 168, 24
ENC_IN, DEC_IN, HID = 32, 16, 512
G = 4 * HID  # 2048
NCORES = 8
BL = B // NCORES  # 128 batch per core
XCHUNK = 28  # encoder-input steps per DMA chunk

WSCALE = 16.0  # fp8 weight pre-scale
HSCALE = 16.0  # fp8 h pre-scale
GSCALE = 1.0 / (WSCALE * HSCALE)  # ACT de-scale on gate reads

# gate reorder: torch order [i, f, g, o] -> [g, i, f, o]
_PERM = np.concatenate([np.arange(1024, 1536), np.arange(0, 512),
                        np.arange(512, 1024), np.arange(1536, 2048)])


def _bf16(x):
    return np.ascontiguousarray(x.astype(ml_dtypes.bfloat16))


def _fp8(x):
    return np.ascontiguousarray(
        np.clip(x, -224.0, 224.0).astype(ml_dtypes.float8_e4m3))


def _f32(x):
    return np.ascontiguousarray(x.astype(np.float32))


def _wT_kxn(W, conv=_bf16, scale=1.0):
    """[4H, D] gate-major weight -> reordered W.T as [128, D//128, 4H]."""
    Wt = W[_PERM].T * scale  # [D, 2048]
    D = Wt.shape[0]
    return conv(Wt.reshape(D // 128, 128, G).transpose(1, 0, 2))


def build_kernel(T=T_ENC, HD=H_DEC):
    nc = bacc.Bacc("TRN2", target_bir_lowering=False, debug=False,
                   num_devices=NCORES)

    def din(name, shape, dt):
        return nc.dram_tensor(name, shape, dt, kind="ExternalInput").ap()

    x_d = din("x", [ENC_IN + 1, T, BL], BF16)        # enc features + ones row
    w0_d = din("w0", [ENC_IN + 1, G], BF16)           # (W_ih0T + bias row)*256
    wh0_d = din("wh0", [128, 4, G], FP8)              # *16
    wi1_d = din("wi1", [128, 4, G], FP8)              # *16
    wh1_d = din("wh1", [128, 4, G], FP8)              # *16
    wctx_d = din("wctx", [128, 4, G], BF16)
    whd_d = din("whd", [128, 4, G], BF16)
    be_d = din("be", [33, G + 128], BF16)  # row0: b1*256|ones, row32: bd|ones
    covy_d = din("covy", [DEC_IN + 1, HD, BL], BF16)  # dec covariates + y_prev
    wcy_d = din("wcy", [DEC_IN + 1, G], BF16)
    # head weights broadcast across partitions + per-partition biases:
    # cols 0:512 W_mu, 512:1024 W_sig, 1024 b_mu, 1025 b_sig
    wms_d = din("wms", [128, 2 * HID + 2], F32)

    mu_d = nc.dram_tensor("mu", [BL, HD], F32, kind="ExternalOutput").ap()
    sg_d = nc.dram_tensor("sg", [BL, HD], F32, kind="ExternalOutput").ap()

    with tile.TileContext(nc) as tc:
        _emit(tc, T, HD, x_d, w0_d, wh0_d, wi1_d, wh1_d, wctx_d, whd_d,
              be_d, covy_d, wcy_d, wms_d, mu_d, sg_d)
    nc.compile()
    return nc


def _emit(tc, T, HD, x_d, w0_d, wh0_d, wi1_d, wh1_d, wctx_d, whd_d,
          be_d, covy_d, wcy_d, wms_d, mu_d, sg_d):
    nc = tc.nc
    mm = nc.tensor.matmul

    with (
        tc.tile_pool(name="const", bufs=1) as cp,
        tc.tile_pool(name="xp", bufs=2) as xp,
        tc.tile_pool(name="sig", bufs=3) as sigp,
        tc.tile_pool(name="small", bufs=3) as smp,
        tc.tile_pool(name="hp", bufs=2) as hp,
        tc.tile_pool(name="htp", bufs=3) as htp,
        tc.tile_pool(name="ht8p", bufs=4) as ht8p,
        tc.tile_pool(name="psum", bufs=2, space="PSUM") as pp,
    ):
        # ---- persistent tiles / weight loads ----
        def load(name, dram, shape, dt):
            t = cp.tile(shape, dt, tag=name)
            nc.sync.dma_start(t[:], dram[:])
            return t

        w0 = load("w0", w0_d, [ENC_IN + 1, G], BF16)
        wh0 = load("wh0", wh0_d, [128, 4, G], FP8)
        be = load("be", be_d, [33, G + 128], BF16)
        wi1 = load("wi1", wi1_d, [128, 4, G], FP8)
        wh1 = load("wh1", wh1_d, [128, 4, G], FP8)

        ident = cp.tile([128, 128], BF16, tag="ident")
        make_identity(nc, ident[:])

        ones_r = be[0:1, G:G + 128]
        ones32_r = be[32:33, G:G + 128]
        b1_r = be[0:1, 0:G]
        bd_r = be[32:33, 0:G]

        c0 = cp.tile([128, HID], F32, tag="c0")
        c1 = cp.tile([128, HID], F32, tag="c1")
        cd = cp.tile([128, HID], F32, tag="cd")
        mu_b = cp.tile([128, HD], F32, tag="mu_b")
        sp_b = cp.tile([128, HD], F32, tag="sp_b")
        sg_b = cp.tile([128, HD], F32, tag="sg_b")

        NS = G // 512  # 4 n-chunks

        def cell(g, c, first, h_tag, scale=1.0):
            """gates psum [g|i|f|o] -> h (bf16 [128, HID]) via ACT/DVE/Pool.

            ACT: 3 ops only (tanh(g), one merged sigmoid(i,f,o), tanh(c)) —
            ACT is the near-saturated engine. The f*c product runs on the
            otherwise-idle Pool engine so DVE's serial m1->add chain shrinks.
            """
            tg = smp.tile([128, HID], F32, tag="tg")
            nc.scalar.activation(tg[:], g[:, 0:HID], AF.Tanh, scale=scale)
            sio = sigp.tile([128, 3 * HID], F32, tag="sio")
            nc.scalar.activation(sio[:], g[:, HID:G], AF.Sigmoid, scale=scale)
            si = sio[:, 0:HID]
            sf = sio[:, HID:2 * HID]
            so = sio[:, 2 * HID:3 * HID]
            if first:
                nc.vector.tensor_mul(c[:], si, tg[:])
            else:
                m1 = smp.tile([128, HID], F32, tag="m1")
                nc.vector.tensor_mul(m1[:], si, tg[:])
                m2 = smp.tile([128, HID], F32, tag="m2")
                nc.gpsimd.tensor_mul(m2[:], sf, c[:])
                nc.vector.tensor_add(c[:], m1[:], m2[:])
            tcn = smp.tile([128, HID], F32, tag="tc")
            nc.scalar.activation(tcn[:], c[:], AF.Tanh)
            h = hp.tile([128, HID], BF16, tag=h_tag)
            hh = HID // 2
            nc.vector.tensor_mul(h[:, 0:hh], so[:, 0:hh], tcn[:, 0:hh])
            nc.vector.tensor_mul(h[:, hh:HID], so[:, hh:HID], tcn[:, hh:HID])
            return h

        def transp(h, tag):
            """single-instruction [128,512] -> [128,4,128] xbar transpose."""
            ht = htp.tile([128, 4, 128], BF16, tag=tag)
            nc.sync.dma_start_transpose(ht[:], h[:])
            return ht

        def cast8(ht, tag):
            """bf16 hT -> fp8 (x16) on DVE. Emission point chosen so the
            strict DVE FIFO never idle-waits on the transpose DMA."""
            ht8 = ht8p.tile([128, 4, 128], FP8, tag=tag)
            nc.vector.tensor_scalar_mul(ht8[:], ht[:], HSCALE)
            return ht8

        # ================= encoder =================
        # L1 runs one step behind L0: while L0(t)'s elementwise chain runs
        # on ACT/DVE/DMA, the PE stays busy on L1(t-1)'s matmuls.
        h0T8_hist = {}
        h1T8 = None
        h1T = None
        h1T_pending = None  # bf16 transposed h1 awaiting its DVE fp8 cast

        x_cur = None
        x_nxt = None

        def load_xchunk(t0):
            nxc = min(XCHUNK, T - t0)
            xt = xp.tile([ENC_IN + 1, XCHUNK, BL], BF16, tag="x")
            nc.sync.dma_start(xt[:, :nxc, :], x_d[:, t0:t0 + nxc, :])
            return xt

        for t in range(T):
            if t == 0:
                x_cur = load_xchunk(0)
                if T > XCHUNK:
                    x_nxt = load_xchunk(XCHUNK)
            elif t % XCHUNK == 0:
                x_cur = x_nxt
                if t + XCHUNK < T:
                    x_nxt = load_xchunk(t + XCHUNK)
            ti = t % XCHUNK

            # ---- layer 0 step t: input mms open the 4 psum groups ----
            g0 = pp.tile([128, G], F32, tag="g")
            for n in range(NS):
                s = slice(n * 512, (n + 1) * 512)
                mm(g0[:, s], x_cur[:, ti, :], w0[:, s],
                   start=True, stop=(t == 0))
            # L1(t-1) bias mms: always-ready PE filler while L0's recurrent
            # matmuls wait for the h0T8 transpose+cast.
            g1 = None
            if t >= 1:
                g1 = pp.tile([128, G], F32, tag="g")
                for n in range(NS):
                    s = slice(n * 512, (n + 1) * 512)
                    mm(g1[:, s], ones_r, b1_r[:, s], start=True, stop=False)
            # first DVE op of the iteration: cast last iteration's h1T
            # (its transpose completed around the iteration boundary).
            if h1T_pending is not None:
                h1T8 = cast8(h1T_pending, "h1T8")
                h1T_pending = None
            # L0 recurrent, fp8 DoubleRow, n-outer so chunk n completes early
            if t > 0:
                hp8 = h0T8_hist[t - 1]
                for n in range(NS):
                    s = slice(n * 512, (n + 1) * 512)
                    mm(g0[:, s], hp8[:, 0:2, :], wh0[:, 0:2, s],
                       perf_mode=DR, start=False, stop=False)
                    mm(g0[:, s], hp8[:, 2:4, :], wh0[:, 2:4, s],
                       perf_mode=DR, start=False, stop=True)
            h0 = cell(g0, c0, t == 0, "h0", scale=GSCALE)
            h0T = transp(h0, "h0T")
            # cast right after cell0's DVE ops: DVE briefly waits on the
            # transpose, but h0T8 lands ~3us earlier than an end-of-iteration
            # cast would, unblocking next step's L0-rec matmuls.
            h0T8_hist[t] = cast8(h0T, "h0T8")
            h0T8_hist.pop(t - 2, None)

            # ---- layer 1, step t-1 ----
            if t >= 1:
                tp = t - 1
                hp8 = h0T8_hist[tp]
                for n in range(NS):
                    s = slice(n * 512, (n + 1) * 512)
                    mm(g1[:, s], hp8[:, 0:2, :], wi1[:, 0:2, s],
                       perf_mode=DR, start=False, stop=False)
                    mm(g1[:, s], hp8[:, 2:4, :], wi1[:, 2:4, s],
                       perf_mode=DR, start=False, stop=(tp == 0))
                    if tp > 0:
                        mm(g1[:, s], h1T8[:, 0:2, :], wh1[:, 0:2, s],
                           perf_mode=DR, start=False, stop=False)
                        mm(g1[:, s], h1T8[:, 2:4, :], wh1[:, 2:4, s],
                           perf_mode=DR, start=False, stop=True)
                h1 = cell(g1, c1, tp == 0, "h1", scale=GSCALE)
                h1T = transp(h1, "h1T")
                h1T_pending = h1T

        # final L1 step (t = T-1)
        g1 = pp.tile([128, G], F32, tag="g")
        for n in range(NS):
            s = slice(n * 512, (n + 1) * 512)
            mm(g1[:, s], ones_r, b1_r[:, s], start=True, stop=False)
        h1T8 = cast8(h1T_pending, "h1T8")
        hp8 = h0T8_hist[T - 1]
        for n in range(NS):
            s = slice(n * 512, (n + 1) * 512)
            mm(g1[:, s], hp8[:, 0:2, :], wi1[:, 0:2, s],
               perf_mode=DR, start=False, stop=False)
            mm(g1[:, s], hp8[:, 2:4, :], wi1[:, 2:4, s],
               perf_mode=DR, start=False, stop=False)
            mm(g1[:, s], h1T8[:, 0:2, :], wh1[:, 0:2, s],
               perf_mode=DR, start=False, stop=False)
            mm(g1[:, s], h1T8[:, 2:4, :], wh1[:, 2:4, s],
               perf_mode=DR, start=False, stop=True)
        h1 = cell(g1, c1, False, "h1", scale=GSCALE)
        h1T = transp(h1, "h1T")

        # ================= decoder (bf16) =================
        wctx = load("wctx", wctx_d, [128, 4, G], BF16)
        whd = load("whd", whd_d, [128, 4, G], BF16)
        covy = load("covy", covy_d, [DEC_IN + 1, HD, BL], BF16)
        wcy = load("wcy", wcy_d, [DEC_IN + 1, G], BF16)
        wms = load("wms", wms_d, [128, 2 * HID + 2], F32)
        # one-time: ctx_pre = context @ W_ctx.T + (b_ihd + b_hhd)
        cps = pp.tile([128, G], F32, tag="g")
        for n in range(NS):
            s = slice(n * 512, (n + 1) * 512)
            mm(cps[:, s], ones32_r, bd_r[:, s], start=True, stop=False)
        for k in range(4):
            for n in range(NS):
                s = slice(n * 512, (n + 1) * 512)
                mm(cps[:, s], h1T[:, k, :], wctx[:, k, s],
                   start=False, stop=(k == 3))
        ctxp = cp.tile([128, G], BF16, tag="ctxp")
        nc.scalar.copy(ctxp[:], cps[:])

        hdT = None
        for t in range(HD):
            gd = pp.tile([128, G], F32, tag="g")
            for n in range(NS):
                s = slice(n * 512, (n + 1) * 512)
                mm(gd[:, s], ident[:], ctxp[:, s], start=True, stop=False)
                mm(gd[:, s], covy[:, t, :], wcy[:, s],
                   start=False, stop=(t == 0))
            if t > 0:
                for k in range(4):
                    for n in range(NS):
                        s = slice(n * 512, (n + 1) * 512)
                        mm(gd[:, s], hdT[:, k, :], whd[:, k, s],
                           start=False, stop=(k == 3))
            hd = cell(gd, cd, t == 0, "hd")
            hdT = transp(hd, "hdT")

            # heads: mu/sigma dot-products on DVE, off the critical path
            hsc = smp.tile([128, HID], F32, tag="hsc")
            nc.vector.scalar_tensor_tensor(
                hsc[:], hd[:], 1.0, wms[:, 0:HID],
                op0=ALU.mult, op1=ALU.mult, accum_out=mu_b[:, t:t + 1])
            hsc2 = smp.tile([128, HID], F32, tag="hsc2")
            nc.vector.scalar_tensor_tensor(
                hsc2[:], hd[:], 1.0, wms[:, HID:2 * HID],
                op0=ALU.mult, op1=ALU.mult, accum_out=sp_b[:, t:t + 1])

        # add head biases; sigma = softplus(x) + 1e-6 via ln(exp(x)+1)
        nc.vector.tensor_scalar_add(mu_b[:], mu_b[:],
                                    wms[:, 2 * HID:2 * HID + 1])
        nc.vector.tensor_scalar_add(sp_b[:], sp_b[:],
                                    wms[:, 2 * HID + 1:2 * HID + 2])
        nc.scalar.activation(sp_b[:], sp_b[:], AF.Exp)
        nc.scalar.activation(sg_b[:], sp_b[:], AF.Ln, bias=1.0)
        nc.vector.tensor_scalar_add(sg_b[:], sg_b[:], 1e-6)
        nc.sync.dma_start(mu_d[:], mu_b[:])
        nc.sync.dma_start(sg_d[:], sg_b[:])


def _make_be(b1, bdv):
    be = np.zeros((33, G + 128), np.float32)
    be[0, :G] = b1 / GSCALE  # *256: de-scaled by the ACT gate reads
    be[32, :G] = bdv
    be[0, G:] = 1.0
    be[32, G:] = 1.0
    return _bf16(be)


def _make_wms(W_mu, W_sig, b_mu, b_sig):
    w = np.zeros((128, 2 * HID + 2), np.float32)
    w[:, 0:HID] = W_mu[0][None, :]
    w[:, HID:2 * HID] = W_sig[0][None, :]
    w[:, 2 * HID] = b_mu[0]
    w[:, 2 * HID + 1] = b_sig[0]
    return _f32(w)


def prep_inputs(inputs, T=T_ENC, HD=H_DEC):
    """Full-batch inputs -> list of per-core input maps (host layout prep)."""
    enc = _f32(np.asarray(inputs["enc_inp"]))[:, :T]
    dec = _f32(np.asarray(inputs["dec_inp"]))[:, :HD]
    tgt = _f32(np.asarray(inputs["tgt"]))[:, :HD]

    W_ih0, W_hh0 = np.asarray(inputs["W_ih0"]), np.asarray(inputs["W_hh0"])
    W_ih1, W_hh1 = np.asarray(inputs["W_ih1"]), np.asarray(inputs["W_hh1"])
    W_ihd, W_hhd = np.asarray(inputs["W_ihd"]), np.asarray(inputs["W_hhd"])
    b0 = _f32(np.asarray(inputs["b_ih0"]) + np.asarray(inputs["b_hh0"]))[_PERM]
    b1 = _f32(np.asarray(inputs["b_ih1"]) + np.asarray(inputs["b_hh1"]))[_PERM]
    bdv = _f32(np.asarray(inputs["b_ihd"]) + np.asarray(inputs["b_hhd"]))[_PERM]
    W_mu, b_mu = np.asarray(inputs["W_mu"]), np.asarray(inputs["b_mu"])
    W_sig, b_sig = np.asarray(inputs["W_sig"]), np.asarray(inputs["b_sig"])

    # x-side weights *256 in bf16 (exact power-of-two scale); gate reads
    # apply scale=1/256. b0 rides the ones-row of x.
    w0 = np.concatenate([W_ih0[_PERM].T, b0[None, :]], 0) / GSCALE  # [33,2048]
    shared = {
        "w0": _bf16(w0),
        "wh0": _wT_kxn(W_hh0, conv=_fp8, scale=WSCALE),
        "wi1": _wT_kxn(W_ih1, conv=_fp8, scale=WSCALE),
        "wh1": _wT_kxn(W_hh1, conv=_fp8, scale=WSCALE),
        "wctx": _wT_kxn(W_ihd[:, DEC_IN:DEC_IN + HID]),
        "whd": _wT_kxn(W_hhd),
        "be": _make_be(b1, bdv),
        "wcy": _bf16(np.concatenate(
            [W_ihd[_PERM][:, :DEC_IN].T, W_ihd[_PERM][:, DEC_IN + HID:].T], 0)),
        "wms": _make_wms(W_mu, W_sig, b_mu, b_sig),
    }

    in_maps = []
    for c in range(NCORES):
        sl = slice(c * BL, (c + 1) * BL)
        xe = np.ones((ENC_IN + 1, T, BL), np.float32)
        xe[:ENC_IN] = enc[sl].transpose(2, 1, 0)
        cy = np.zeros((DEC_IN + 1, HD, BL), np.float32)
        cy[:DEC_IN] = dec[sl].transpose(2, 1, 0)
        cy[DEC_IN, 1:] = tgt[sl, :HD - 1].T
        m = dict(shared)
        m["x"] = _bf16(xe)
        m["covy"] = _bf16(cy)
        in_maps.append(m)
    return in_maps


_NC_CACHE = {}


def _get_nc(T=T_ENC, HD=H_DEC):
    key = (T, HD)
    if key not in _NC_CACHE:
        _NC_CACHE[key] = build_kernel(T, HD)
    return _NC_CACHE[key]


def run(inputs, T=T_ENC, HD=H_DEC, **kw):
    nc = _get_nc(T, HD)
    in_maps = prep_inputs(inputs, T, HD)
    res = run_bass_kernel_spmd(nc, in_maps, core_ids=list(range(NCORES)), **kw)
    mu = np.concatenate([res.results[c]["mu"] for c in range(NCORES)], 0)
    sg = np.concatenate([res.results[c]["sg"] for c in range(NCORES)], 0)
    return (mu, sg), res


def kernel(**inputs):
    (mu, sg), _ = run(inputs)
    return mu, sg


# revision 15
# speedup vs baseline: 1.9466x; 1.2711x over previous
"""DeepAR (2-layer LSTM encoder + LSTM-cell decoder) Trainium2 Bass kernel.

Sharding: pure data parallel, batch 1024 -> 128 per core across 8 cores
(batch 128 == SBUF partition width).

Per-core design (v2 — fp8 DoubleRow encoder):
  - gates in [128 batch, 2048 gate] layout, gate order reordered to
    [g, i, f, o]: tanh(g) finishes first so the DVE chain starts early,
    and one sigmoid covers cols 512:2048.
  - encoder recurrent matmuls (h@W_hh0, h0@W_ih1, h1@W_hh1) run in
    fp8e4 with perf_mode=DoubleRow: 2 fp8 weights per PE cell -> ~2x
    matmul throughput. Weights and h both pre-scaled by 16 (so fp8
    mantissa sees normal-range values); the x-side weights/biases are
    scaled by 256 in bf16, and the cell activations apply scale=1/256.
  - h produced in bf16, transposed to stationary [K,M] layout with a
    SINGLE [128,512]->[128,4,128] xbar transpose per h (chunk-major
    layout verified on HW), then cast+scaled to fp8 on the idle Pool
    engine (keeps DVE/ACT FIFOs clean).
  - per-512-col n-chunk accumulation groups emitted so each PSUM bank
    completes early and ACT starts before the whole gate tensor is done.
  - layer 1 runs one step behind layer 0 so the PE always has
    independent matmul work while layer 0's elementwise chain runs.
  - decoder kept in bf16 (24 steps, accuracy headroom): context
    contribution precomputed once and injected into PSUM via identity
    matmul each step; mu/sigma heads are DVE dot-products.
"""
import numpy as np
import ml_dtypes

import concourse.bass as bass
import concourse.mybir as mybir
import concourse.tile as tile
from concourse import bacc
from concourse.bass_utils import run_bass_kernel_spmd
from concourse.masks import make_identity

F32 = mybir.dt.float32
BF16 = mybir.dt.bfloat16
FP8 = mybir.dt.float8e4
AF = mybir.ActivationFunctionType
ALU = mybir.AluOpType
DR = mybir.MatmulPerfMode.DoubleRow

B, T_ENC, H_DEC = 1024, 168, 24
ENC_IN, DEC_IN, HID = 32, 16, 512
G = 4 * HID  # 2048
NCORES = 8
BL = B // NCORES  # 128 batch per core
XCHUNK = 28  # encoder-input steps per DMA chunk

WSCALE = 16.0  # fp8 weight pre-scale
HSCALE = 16.0  # fp8 h pre-scale
GSCALE = 1.0 / (WSCALE * HSCALE)  # ACT de-scale on gate reads

# gate reorder: torch order [i, f, g, o] -> [g, i, f, o]
_PERM = np.concatenate([np.arange(1024, 1536), np.arange(0, 512),
                        np.arange(512, 1024), np.arange(1536, 2048)])


def _bf16(x):
    return np.ascontiguousarray(x.astype(ml_dtypes.bfloat16))


def _fp8(x):
    return np.ascontiguousarray(
        np.clip(x, -224.0, 224.0).astype(ml_dtypes.float8_e4m3))


def _f32(x):
    return np.ascontiguousarray(x.astype(np.float32))


def _wT_kxn(W, conv=_bf16, scale=1.0):
    """[4H, D] gate-major weight -> reordered W.T as [128, D//128, 4H]."""
    Wt = W[_PERM].T * scale  # [D, 2048]
    D = Wt.shape[0]
    return conv(Wt.reshape(D // 128, 128, G).transpose(1, 0, 2))


def build_kernel(T=T_ENC, HD=H_DEC):
    nc = bacc.Bacc("TRN2", target_bir_lowering=False, debug=False,
                   num_devices=NCORES)

    def din(name, shape, dt):
        return nc.dram_tensor(name, shape, dt, kind="ExternalInput").ap()

    x_d = din("x", [ENC_IN + 1, T, BL], BF16)        # enc features + ones row
    w0_d = din("w0", [ENC_IN + 1, G], BF16)           # (W_ih0T + bias row)*256
    wh0_d = din("wh0", [128, 4, G], FP8)              # *16
    wi1_d = din("wi1", [128, 4, G], FP8)              # *16
    wh1_d = din("wh1", [128, 4, G], FP8)              # *16
    wctx_d = din("wctx", [128, 4, G], BF16)
    whd_d = din("whd", [128, 4, G], BF16)
    be_d = din("be", [33, G + 128], BF16)  # row0: b1*256|ones, row32: bd|ones
    covy_d = din("covy", [DEC_IN + 1, HD, BL], BF16)  # dec covariates + y_prev
    wcy_d = din("wcy", [DEC_IN + 1, G], BF16)
    # head weights broadcast across partitions + per-partition biases:
    # cols 0:512 W_mu, 512:1024 W_sig, 1024 b_mu, 1025 b_sig
    wms_d = din("wms", [128, 2 * HID + 2], F32)

    mu_d = nc.dram_tensor("mu", [BL, HD], F32, kind="ExternalOutput").ap()
    sg_d = nc.dram_tensor("sg", [BL, HD], F32, kind="ExternalOutput").ap()

    with tile.TileContext(nc) as tc:
        _emit(tc, T, HD, x_d, w0_d, wh0_d, wi1_d, wh1_d, wctx_d, whd_d,
              be_d, covy_d, wcy_d, wms_d, mu_d, sg_d)
    nc.compile()
    return nc


def _emit(tc, T, HD, x_d, w0_d, wh0_d, wi1_d, wh1_d, wctx_d, whd_d,
          be_d, covy_d, wcy_d, wms_d, mu_d, sg_d):
    nc = tc.nc
    mm = nc.tensor.matmul

    with (
        tc.tile_pool(name="const", bufs=1) as cp,
        tc.tile_pool(name="xp", bufs=2) as xp,
        tc.tile_pool(name="sig", bufs=3) as sigp,
        tc.tile_pool(name="small", bufs=3) as smp,
        tc.tile_pool(name="hp", bufs=3) as hp,
        tc.tile_pool(name="ht8p", bufs=4) as ht8p,
        # gates live as four 1-bank [128,512] chunk tiles: 6 bufs = 6 banks,
        # leaving bank room for the PE-transpose staging tiles below.
        tc.tile_pool(name="psum", bufs=6, space="PSUM") as pp,
        tc.tile_pool(name="htps", bufs=2, space="PSUM") as hTpp,
    ):
        # ---- persistent tiles / weight loads ----
        def load(name, dram, shape, dt):
            t = cp.tile(shape, dt, tag=name)
            nc.sync.dma_start(t[:], dram[:])
            return t

        w0 = load("w0", w0_d, [ENC_IN + 1, G], BF16)
        wh0 = load("wh0", wh0_d, [128, 4, G], FP8)
        be = load("be", be_d, [33, G + 128], BF16)
        wi1 = load("wi1", wi1_d, [128, 4, G], FP8)
        wh1 = load("wh1", wh1_d, [128, 4, G], FP8)

        ident = cp.tile([128, 128], BF16, tag="ident")
        make_identity(nc, ident[:])

        ones_r = be[0:1, G:G + 128]
        ones32_r = be[32:33, G:G + 128]
        b1_r = be[0:1, 0:G]
        bd_r = be[32:33, 0:G]

        c0 = cp.tile([128, HID], F32, tag="c0")
        c1 = cp.tile([128, HID], F32, tag="c1")
        cd = cp.tile([128, HID], F32, tag="cd")
        mu_b = cp.tile([128, HD], F32, tag="mu_b")
        sp_b = cp.tile([128, HD], F32, tag="sp_b")
        sg_b = cp.tile([128, HD], F32, tag="sg_b")

        NS = G // 512  # 4 n-chunks

        def cell(gc, c, first, h_tag, scale=1.0, mid_emit=None):
            """gate chunk tiles [g, i, f, o] (each [128,512] psum) -> h bf16.

            One ACT op per gate chunk so each starts as soon as its chunk's
            matmuls land; the DVE m1/m2/add chain follows chunk arrivals.
            mid_emit() is called between `add` and the h muls — a natural
            DVE idle gap (DVE waits on ACT tanh(c) there) used to slot in
            the previous h1's fp8 cast.
            """
            tg = smp.tile([128, HID], F32, tag="tg")
            nc.scalar.activation(tg[:], gc[0][:], AF.Tanh, scale=scale)
            si = smp.tile([128, HID], F32, tag="si")
            nc.scalar.activation(si[:], gc[1][:], AF.Sigmoid, scale=scale)
            sf = smp.tile([128, HID], F32, tag="sf")
            nc.scalar.activation(sf[:], gc[2][:], AF.Sigmoid, scale=scale)
            so = sigp.tile([128, HID], F32, tag="so")
            nc.scalar.activation(so[:], gc[3][:], AF.Sigmoid, scale=scale)
            if first:
                nc.vector.tensor_mul(c[:], si[:], tg[:])
            else:
                m1 = smp.tile([128, HID], F32, tag="m1")
                nc.vector.tensor_mul(m1[:], si[:], tg[:])
                m2 = smp.tile([128, HID], F32, tag="m2")
                nc.vector.tensor_mul(m2[:], sf[:], c[:])
                nc.vector.tensor_add(c[:], m1[:], m2[:])
            if mid_emit is not None:
                mid_emit()
            tcn = smp.tile([128, HID], F32, tag="tc")
            nc.scalar.activation(tcn[:], c[:], AF.Tanh)
            h = hp.tile([128, HID], BF16, tag=h_tag)
            hh = HID // 2
            nc.vector.tensor_mul(h[:, 0:hh], so[:, 0:hh], tcn[:, 0:hh])
            nc.vector.tensor_mul(h[:, hh:HID], so[:, hh:HID], tcn[:, hh:HID])
            return h

        def pe_transp(h, tag):
            """h [128,512] bf16 SBUF -> [128,4,128] bf16 PSUM via 4 PE
            transposes (~60ns each) — no DMA queue, no 1.7us xbar latency."""
            ht = hTpp.tile([128, 4, 128], BF16, tag=tag)
            for k in range(4):
                nc.tensor.transpose(ht[:, k, :], h[:, k * 128:(k + 1) * 128],
                                    ident[:])
            return ht

        def cast8(ht_ps, tag, scale=HSCALE, dt=FP8):
            """PSUM hT -> fp8 (x16) SBUF on DVE."""
            ht8 = ht8p.tile([128, 4, 128], dt, tag=tag)
            nc.vector.tensor_scalar_mul(ht8[:], ht_ps[:], scale)
            return ht8

        # ================= encoder =================
        # L1 runs one step behind L0: while L0(t)'s elementwise chain runs
        # on ACT/DVE, the PE stays busy on L1(t-1)'s matmuls.
        h0T8_hist = {}
        h1T8 = None
        h1_pending = None  # bf16 h1 awaiting PE transpose + fp8 cast

        x_cur = None
        x_nxt = None

        def load_xchunk(t0):
            nxc = min(XCHUNK, T - t0)
            xt = xp.tile([ENC_IN + 1, XCHUNK, BL], BF16, tag="x")
            nc.sync.dma_start(xt[:, :nxc, :], x_d[:, t0:t0 + nxc, :])
            return xt

        for t in range(T):
            if t == 0:
                x_cur = load_xchunk(0)
                if T > XCHUNK:
                    x_nxt = load_xchunk(XCHUNK)
            elif t % XCHUNK == 0:
                x_cur = x_nxt
                if t + XCHUNK < T:
                    x_nxt = load_xchunk(t + XCHUNK)
            ti = t % XCHUNK

            # ---- layer 0 step t: per-chunk groups [in, DR kp0, DR kp1]
            # so chunk n's ACT op can start as soon as its 3 matmuls land.
            g0c = []
            for n in range(NS):
                s = slice(n * 512, (n + 1) * 512)
                gcn = pp.tile([128, 512], F32, tag="g")
                mm(gcn[:], x_cur[:, ti, :], w0[:, s],
                   start=True, stop=(t == 0))
                if t > 0:
                    hp8 = h0T8_hist[t - 1]
                    mm(gcn[:], hp8[:, 0:2, :], wh0[:, 0:2, s],
                       perf_mode=DR, start=False, stop=False)
                    mm(gcn[:], hp8[:, 2:4, :], wh0[:, 2:4, s],
                       perf_mode=DR, start=False, stop=True)
                g0c.append(gcn)
            # PE: transpose h1(t-2) now — it has been ready since early in
            # this iteration, and the PE reaches this point right after the
            # L0 matmuls.
            h1T_ps = None
            if h1_pending is not None:
                h1T_ps = pe_transp(h1_pending, "hTps")
                h1_pending = None
            # L1(t-1) bias mms: always-ready PE filler, opens the g1 groups.
            g1c = None
            if t >= 1:
                g1c = []
                for n in range(NS):
                    s = slice(n * 512, (n + 1) * 512)
                    gcn = pp.tile([128, 512], F32, tag="g")
                    mm(gcn[:], ones_r, b1_r[:, s], start=True, stop=False)
                    g1c.append(gcn)

            def _cast_h1():
                nonlocal h1T8
                if h1T_ps is not None:
                    h1T8 = cast8(h1T_ps, "h1T8")
            h0 = cell(g0c, c0, t == 0, "h0", scale=GSCALE, mid_emit=_cast_h1)

            # ---- layer 1, step t-1 ----
            if t >= 1:
                tp = t - 1
                hp8 = h0T8_hist[tp]
                for n in range(NS):
                    s = slice(n * 512, (n + 1) * 512)
                    gcn = g1c[n]
                    mm(gcn[:], hp8[:, 0:2, :], wi1[:, 0:2, s],
                       perf_mode=DR, start=False, stop=False)
                    mm(gcn[:], hp8[:, 2:4, :], wi1[:, 2:4, s],
                       perf_mode=DR, start=False, stop=(tp == 0))
                    if tp > 0:
                        mm(gcn[:], h1T8[:, 0:2, :], wh1[:, 0:2, s],
                           perf_mode=DR, start=False, stop=False)
                        mm(gcn[:], h1T8[:, 2:4, :], wh1[:, 2:4, s],
                           perf_mode=DR, start=False, stop=True)
            # PE: transpose h0(t) right after the L1 matmuls (h0 is ready by
            # then); the DVE cast follows cell0's muls in the DVE FIFO.
            h0T_ps = pe_transp(h0, "hTps")
            h0T8_hist[t] = cast8(h0T_ps, "h0T8")
            h0T8_hist.pop(t - 2, None)
            if t >= 1:
                h1 = cell(g1c, c1, tp == 0, "h1", scale=GSCALE)
                h1_pending = h1

        # final L1 step (t = T-1)
        h1T_ps = pe_transp(h1_pending, "hTps")
        h1T8 = cast8(h1T_ps, "h1T8")
        g1c = []
        for n in range(NS):
            s = slice(n * 512, (n + 1) * 512)
            gcn = pp.tile([128, 512], F32, tag="g")
            mm(gcn[:], ones_r, b1_r[:, s], start=True, stop=False)
            g1c.append(gcn)
        hp8 = h0T8_hist[T - 1]
        for n in range(NS):
            s = slice(n * 512, (n + 1) * 512)
            gcn = g1c[n]
            mm(gcn[:], hp8[:, 0:2, :], wi1[:, 0:2, s],
               perf_mode=DR, start=False, stop=False)
            mm(gcn[:], hp8[:, 2:4, :], wi1[:, 2:4, s],
               perf_mode=DR, start=False, stop=False)
            mm(gcn[:], h1T8[:, 0:2, :], wh1[:, 0:2, s],
               perf_mode=DR, start=False, stop=False)
            mm(gcn[:], h1T8[:, 2:4, :], wh1[:, 2:4, s],
               perf_mode=DR, start=False, stop=True)
        h1 = cell(g1c, c1, False, "h1", scale=GSCALE)
        h1T_ps = pe_transp(h1, "hTps")
        h1T = cast8(h1T_ps, "h1T", scale=1.0, dt=BF16)

        # ================= decoder (bf16) =================
        wctx = load("wctx", wctx_d, [128, 4, G], BF16)
        whd = load("whd", whd_d, [128, 4, G], BF16)
        covy = load("covy", covy_d, [DEC_IN + 1, HD, BL], BF16)
        wcy = load("wcy", wcy_d, [DEC_IN + 1, G], BF16)
        wms = load("wms", wms_d, [128, 2 * HID + 2], F32)
        # one-time: ctx_pre = context @ W_ctx.T + (b_ihd + b_hhd)
        cpsc = []
        for n in range(NS):
            s = slice(n * 512, (n + 1) * 512)
            gcn = pp.tile([128, 512], F32, tag="g")
            mm(gcn[:], ones32_r, bd_r[:, s], start=True, stop=False)
            cpsc.append(gcn)
        for k in range(4):
            for n in range(NS):
                s = slice(n * 512, (n + 1) * 512)
                mm(cpsc[n][:], h1T[:, k, :], wctx[:, k, s],
                   start=False, stop=(k == 3))
        ctxp = cp.tile([128, G], BF16, tag="ctxp")
        for n in range(NS):
            s = slice(n * 512, (n + 1) * 512)
            nc.scalar.copy(ctxp[:, s], cpsc[n][:])

        hdT = None
        for t in range(HD):
            gdc = []
            for n in range(NS):
                s = slice(n * 512, (n + 1) * 512)
                gcn = pp.tile([128, 512], F32, tag="g")
                mm(gcn[:], ident[:], ctxp[:, s], start=True, stop=False)
                mm(gcn[:], covy[:, t, :], wcy[:, s],
                   start=False, stop=(t == 0))
                if t > 0:
                    for k in range(4):
                        mm(gcn[:], hdT[:, k, :], whd[:, k, s],
                           start=False, stop=(k == 3))
                gdc.append(gcn)
            hd = cell(gdc, cd, t == 0, "hd")
            hdT_ps = pe_transp(hd, "hTps")
            hdT = cast8(hdT_ps, "hdT", scale=1.0, dt=BF16)

            # heads: mu/sigma dot-products on DVE, off the critical path
            hsc = smp.tile([128, HID], F32, tag="hsc")
            nc.vector.scalar_tensor_tensor(
                hsc[:], hd[:], 1.0, wms[:, 0:HID],
                op0=ALU.mult, op1=ALU.mult, accum_out=mu_b[:, t:t + 1])
            hsc2 = smp.tile([128, HID], F32, tag="hsc2")
            nc.vector.scalar_tensor_tensor(
                hsc2[:], hd[:], 1.0, wms[:, HID:2 * HID],
                op0=ALU.mult, op1=ALU.mult, accum_out=sp_b[:, t:t + 1])

        # add head biases; sigma = softplus(x) + 1e-6 via ln(exp(x)+1)
        nc.vector.tensor_scalar_add(mu_b[:], mu_b[:],
                                    wms[:, 2 * HID:2 * HID + 1])
        nc.vector.tensor_scalar_add(sp_b[:], sp_b[:],
                                    wms[:, 2 * HID + 1:2 * HID + 2])
        nc.scalar.activation(sp_b[:], sp_b[:], AF.Exp)
        nc.scalar.activation(sg_b[:], sp_b[:], AF.Ln, bias=1.0)
        nc.vector.tensor_scalar_add(sg_b[:], sg_b[:], 1e-6)
        nc.sync.dma_start(mu_d[:], mu_b[:])
        nc.sync.dma_start(sg_d[:], sg_b[:])


def _make_be(b1, bdv):
    be = np.zeros((33, G + 128), np.float32)
    be[0, :G] = b1 / GSCALE  # *256: de-scaled by the ACT gate reads
    be[32, :G] = bdv
    be[0, G:] = 1.0
    be[32, G:] = 1.0
    return _bf16(be)


def _make_wms(W_mu, W_sig, b_mu, b_sig):
    w = np.zeros((128, 2 * HID + 2), np.float32)
    w[:, 0:HID] = W_mu[0][None, :]
    w[:, HID:2 * HID] = W_sig[0][None, :]
    w[:, 2 * HID] = b_mu[0]
    w[:, 2 * HID + 1] = b_sig[0]
    return _f32(w)


def prep_inputs(inputs, T=T_ENC, HD=H_DEC):
    """Full-batch inputs -> list of per-core input maps (host layout prep)."""
    enc = _f32(np.asarray(inputs["enc_inp"]))[:, :T]
    dec = _f32(np.asarray(inputs["dec_inp"]))[:, :HD]
    tgt = _f32(np.asarray(inputs["tgt"]))[:, :HD]

    W_ih0, W_hh0 = np.asarray(inputs["W_ih0"]), np.asarray(inputs["W_hh0"])
    W_ih1, W_hh1 = np.asarray(inputs["W_ih1"]), np.asarray(inputs["W_hh1"])
    W_ihd, W_hhd = np.asarray(inputs["W_ihd"]), np.asarray(inputs["W_hhd"])
    b0 = _f32(np.asarray(inputs["b_ih0"]) + np.asarray(inputs["b_hh0"]))[_PERM]
    b1 = _f32(np.asarray(inputs["b_ih1"]) + np.asarray(inputs["b_hh1"]))[_PERM]
    bdv = _f32(np.asarray(inputs["b_ihd"]) + np.asarray(inputs["b_hhd"]))[_PERM]
    W_mu, b_mu = np.asarray(inputs["W_mu"]), np.asarray(inputs["b_mu"])
    W_sig, b_sig = np.asarray(inputs["W_sig"]), np.asarray(inputs["b_sig"])

    # x-side weights *256 in bf16 (exact power-of-two scale); gate reads
    # apply scale=1/256. b0 rides the ones-row of x.
    w0 = np.concatenate([W_ih0[_PERM].T, b0[None, :]], 0) / GSCALE  # [33,2048]
    shared = {
        "w0": _bf16(w0),
        "wh0": _wT_kxn(W_hh0, conv=_fp8, scale=WSCALE),
        "wi1": _wT_kxn(W_ih1, conv=_fp8, scale=WSCALE),
        "wh1": _wT_kxn(W_hh1, conv=_fp8, scale=WSCALE),
        "wctx": _wT_kxn(W_ihd[:, DEC_IN:DEC_IN + HID]),
        "whd": _wT_kxn(W_hhd),
        "be": _make_be(b1, bdv),
        "wcy": _bf16(np.concatenate(
            [W_ihd[_PERM][:, :DEC_IN].T, W_ihd[_PERM][:, DEC_IN + HID:].T], 0)),
        "wms": _make_wms(W_mu, W_sig, b_mu, b_sig),
    }

    in_maps = []
    for c in range(NCORES):
        sl = slice(c * BL, (c + 1) * BL)
        xe = np.ones((ENC_IN + 1, T, BL), np.float32)
        xe[:ENC_IN] = enc[sl].transpose(2, 1, 0)
        cy = np.zeros((DEC_IN + 1, HD, BL), np.float32)
        cy[:DEC_IN] = dec[sl].transpose(2, 1, 0)
        cy[DEC_IN, 1:] = tgt[sl, :HD - 1].T
        m = dict(shared)
        m["x"] = _bf16(xe)
        m["covy"] = _bf16(cy)
        in_maps.append(m)
    return in_maps


_NC_CACHE = {}


def _get_nc(T=T_ENC, HD=H_DEC):
    key = (T, HD)
    if key not in _NC_CACHE:
        _NC_CACHE[key] = build_kernel(T, HD)
    return _NC_CACHE[key]


def run(inputs, T=T_ENC, HD=H_DEC, **kw):
    nc = _get_nc(T, HD)
    in_maps = prep_inputs(inputs, T, HD)
    res = run_bass_kernel_spmd(nc, in_maps, core_ids=list(range(NCORES)), **kw)
    mu = np.concatenate([res.results[c]["mu"] for c in range(NCORES)], 0)
    sg = np.concatenate([res.results[c]["sg"] for c in range(NCORES)], 0)
    return (mu, sg), res


def kernel(**inputs):
    (mu, sg), _ = run(inputs)
    return mu, sg


# revision 26
# speedup vs baseline: 2.4625x; 1.2650x over previous
"""DeepAR (2-layer LSTM encoder + LSTM-cell decoder) Trainium2 Bass kernel.

Sharding: pure data parallel, batch 1024 -> 128 per core across 8 cores
(batch 128 == SBUF partition width).

Per-core design (v2 — fp8 DoubleRow encoder):
  - gates in [128 batch, 2048 gate] layout, gate order reordered to
    [g, i, f, o]: tanh(g) finishes first so the DVE chain starts early,
    and one sigmoid covers cols 512:2048.
  - encoder recurrent matmuls (h@W_hh0, h0@W_ih1, h1@W_hh1) run in
    fp8e4 with perf_mode=DoubleRow: 2 fp8 weights per PE cell -> ~2x
    matmul throughput. Weights and h both pre-scaled by 16 (so fp8
    mantissa sees normal-range values); the x-side weights/biases are
    scaled by 256 in bf16, and the cell activations apply scale=1/256.
  - h produced in bf16, transposed to stationary [K,M] layout with a
    SINGLE [128,512]->[128,4,128] xbar transpose per h (chunk-major
    layout verified on HW), then cast+scaled to fp8 on the idle Pool
    engine (keeps DVE/ACT FIFOs clean).
  - per-512-col n-chunk accumulation groups emitted so each PSUM bank
    completes early and ACT starts before the whole gate tensor is done.
  - layer 1 runs one step behind layer 0 so the PE always has
    independent matmul work while layer 0's elementwise chain runs.
  - decoder kept in bf16 (24 steps, accuracy headroom): context
    contribution precomputed once and injected into PSUM via identity
    matmul each step; mu/sigma heads are DVE dot-products.
"""
import numpy as np
import ml_dtypes

import concourse.bass as bass
import concourse.mybir as mybir
import concourse.tile as tile
from concourse import bacc
from concourse.bass_utils import run_bass_kernel_spmd
from concourse.masks import make_identity

F32 = mybir.dt.float32
BF16 = mybir.dt.bfloat16
FP8 = mybir.dt.float8e4
AF = mybir.ActivationFunctionType
ALU = mybir.AluOpType
DR = mybir.MatmulPerfMode.DoubleRow

B, T_ENC, H_DEC = 1024, 168, 24
ENC_IN, DEC_IN, HID = 32, 16, 512
G = 4 * HID  # 2048
NCORES = 8
BL = B // NCORES  # 128 batch per core
XCHUNK = 28  # encoder-input steps per DMA chunk

WSCALE = 16.0  # fp8 weight pre-scale
HSCALE = 16.0  # fp8 h pre-scale
GSCALE = 1.0 / (WSCALE * HSCALE)  # ACT de-scale on gate reads

# gate reorder: torch order [i, f, g, o] -> [g, i, f, o]
_PERM = np.concatenate([np.arange(1024, 1536), np.arange(0, 512),
                        np.arange(512, 1024), np.arange(1536, 2048)])


def _bf16(x):
    return np.ascontiguousarray(x.astype(ml_dtypes.bfloat16))


def _fp8(x):
    return np.ascontiguousarray(
        np.clip(x, -224.0, 224.0).astype(ml_dtypes.float8_e4m3))


def _f32(x):
    return np.ascontiguousarray(x.astype(np.float32))


def _wT_kxn(W, conv=_bf16, scale=1.0):
    """[4H, D] gate-major weight -> reordered W.T as [128, D//128, 4H]."""
    Wt = W[_PERM].T * scale  # [D, 2048]
    D = Wt.shape[0]
    return conv(Wt.reshape(D // 128, 128, G).transpose(1, 0, 2))


def build_kernel(T=T_ENC, HD=H_DEC):
    nc = bacc.Bacc("TRN2", target_bir_lowering=False, debug=False,
                   num_devices=NCORES)

    def din(name, shape, dt):
        return nc.dram_tensor(name, shape, dt, kind="ExternalInput").ap()

    x_d = din("x", [ENC_IN + 1, T, BL], BF16)        # enc features + ones row
    w0_d = din("w0", [128, G], BF16)  # (W_ih0T + bias row)*256 @ parts 0,64
    wh0_d = din("wh0", [128, 4, G], FP8)              # *16
    wi1_d = din("wi1", [128, 4, G], FP8)              # *16
    wh1_d = din("wh1", [128, 4, G], FP8)              # *16
    wctx_d = din("wctx", [128, 4, G], BF16)
    whd_d = din("whd", [128, 4, G], BF16)
    be_d = din("be", [33, G + 128], BF16)  # row32: bd|ones (decoder)
    # b1*256 by chunk at partitions 0/32/64/96: cols 0:128 ones, 128:640 bias
    b14_d = din("b14", [128, 640], BF16)
    covy_d = din("covy", [128, HD, BL], BF16)  # dec cov+y at parts 0/32/64/96
    wcy_d = din("wcy", [128, G], BF16)         # replicated at parts 0/32/64/96
    # head weights broadcast across partitions + per-partition biases:
    # cols 0:512 W_mu, 512:1024 W_sig, 1024 b_mu, 1025 b_sig
    wms_d = din("wms", [128, 2 * HID + 2], F32)

    mu_d = nc.dram_tensor("mu", [BL, HD], F32, kind="ExternalOutput").ap()
    sg_d = nc.dram_tensor("sg", [BL, HD], F32, kind="ExternalOutput").ap()

    with tile.TileContext(nc) as tc:
        _emit(tc, T, HD, x_d, w0_d, wh0_d, wi1_d, wh1_d, wctx_d, whd_d,
              be_d, b14_d, covy_d, wcy_d, wms_d, mu_d, sg_d)
    nc.compile()
    return nc


def _emit(tc, T, HD, x_d, w0_d, wh0_d, wi1_d, wh1_d, wctx_d, whd_d,
          be_d, b14_d, covy_d, wcy_d, wms_d, mu_d, sg_d):
    nc = tc.nc
    mm = nc.tensor.matmul

    with (
        tc.tile_pool(name="const", bufs=1) as cp,
        tc.tile_pool(name="xp", bufs=2) as xp,
        tc.tile_pool(name="sig", bufs=3) as sigp,
        tc.tile_pool(name="small", bufs=3) as smp,
        tc.tile_pool(name="hp", bufs=3) as hp,
        tc.tile_pool(name="htp", bufs=3) as htp,
        tc.tile_pool(name="ht8p", bufs=4) as ht8p,
        # gates live as four 1-bank [128,512] chunk tiles: 6 bufs = 6 banks,
        # leaving bank room for the PE-transpose staging tiles below.
        tc.tile_pool(name="psum", bufs=6, space="PSUM") as pp,
        tc.tile_pool(name="htps", bufs=2, space="PSUM") as hTpp,
    ):
        # ---- persistent tiles / weight loads ----
        def load(name, dram, shape, dt):
            t = cp.tile(shape, dt, tag=name)
            nc.sync.dma_start(t[:], dram[:])
            return t

        w0 = load("w0", w0_d, [128, G], BF16)
        wh0 = load("wh0", wh0_d, [128, 4, G], FP8)
        be = load("be", be_d, [33, G + 128], BF16)
        b14 = load("b14", b14_d, [128, 640], BF16)
        wi1 = load("wi1", wi1_d, [128, 4, G], FP8)
        wh1 = load("wh1", wh1_d, [128, 4, G], FP8)

        ident = cp.tile([128, 128], BF16, tag="ident")
        make_identity(nc, ident[:])

        ones32_r = be[32:33, G:G + 128]
        bd_r = be[32:33, 0:G]

        c0 = cp.tile([128, HID], F32, tag="c0")
        c1 = cp.tile([128, HID], F32, tag="c1")
        cd = cp.tile([128, HID], F32, tag="cd")
        mu_b = cp.tile([128, HD], F32, tag="mu_b")
        sp_b = cp.tile([128, HD], F32, tag="sp_b")
        sg_b = cp.tile([128, HD], F32, tag="sg_b")

        NS = G // 512  # 4 n-chunks

        def cell(gc, c, first, h_tag, scale=1.0, mid_emit=None):
            """gate chunk tiles [g, i, f, o] (each [128,512] psum) -> h bf16.

            One ACT op per gate chunk so each starts as soon as its chunk's
            matmuls land; the DVE m1/m2/add chain follows chunk arrivals.
            mid_emit() is called between `add` and the h muls — a natural
            DVE idle gap (DVE waits on ACT tanh(c) there) used to slot in
            the previous h1's fp8 cast.
            """
            tg = smp.tile([128, HID], F32, tag="tg")
            nc.scalar.activation(tg[:], gc[0][:], AF.Tanh, scale=scale)
            si = smp.tile([128, HID], F32, tag="si")
            nc.scalar.activation(si[:], gc[1][:], AF.Sigmoid, scale=scale)
            sf = smp.tile([128, HID], F32, tag="sf")
            nc.scalar.activation(sf[:], gc[2][:], AF.Sigmoid, scale=scale)
            so = sigp.tile([128, HID], F32, tag="so")
            nc.scalar.activation(so[:], gc[3][:], AF.Sigmoid, scale=scale)
            if first:
                nc.vector.tensor_mul(c[:], si[:], tg[:])
            else:
                m1 = smp.tile([128, HID], F32, tag="m1")
                nc.vector.tensor_mul(m1[:], si[:], tg[:])
                m2 = smp.tile([128, HID], F32, tag="m2")
                nc.vector.tensor_mul(m2[:], sf[:], c[:])
                nc.vector.tensor_add(c[:], m1[:], m2[:])
            if mid_emit is not None:
                mid_emit()
            tcn = smp.tile([128, HID], F32, tag="tc")
            nc.scalar.activation(tcn[:], c[:], AF.Tanh)
            h = hp.tile([128, HID], BF16, tag=h_tag)
            hh = HID // 2
            nc.vector.tensor_mul(h[:, 0:hh], so[:, 0:hh], tcn[:, 0:hh])
            nc.vector.tensor_mul(h[:, hh:HID], so[:, hh:HID], tcn[:, hh:HID])
            return h

        def pe_transp(h, tag):
            """h [128,512] bf16 SBUF -> [128,4,128] bf16 PSUM via 4 PE
            transposes (~60ns each) — no DMA queue, no 1.7us xbar latency."""
            ht = hTpp.tile([128, 4, 128], BF16, tag=tag)
            for k in range(4):
                nc.tensor.transpose(ht[:, k, :], h[:, k * 128:(k + 1) * 128],
                                    ident[:])
            return ht

        def cast8(ht_ps, tag, scale=HSCALE, dt=FP8):
            """PSUM hT -> fp8 (x16) SBUF on DVE."""
            ht8 = ht8p.tile([128, 4, 128], dt, tag=tag)
            nc.vector.tensor_scalar_mul(ht8[:], ht_ps[:], scale)
            return ht8

        # ================= encoder =================
        # L1 runs one step behind L0: while L0(t)'s elementwise chain runs
        # on ACT/DVE, the PE stays busy on L1(t-1)'s matmuls.
        h0T8_hist = {}
        h1T8 = None
        h1T_pending = None  # bf16 transposed h1 (SBUF) awaiting fp8 cast

        x_cur = None
        x_nxt = None

        def load_xchunk(t0):
            """x replicated at partitions 0 and 64 for 2-way row tiling."""
            nxc = min(XCHUNK, T - t0)
            xt = xp.tile([128, XCHUNK, BL], BF16, tag="x")
            nc.sync.dma_start(xt[0:ENC_IN + 1, :nxc, :], x_d[:, t0:t0 + nxc, :])
            nc.sync.dma_start(xt[64:64 + ENC_IN + 1, :nxc, :],
                              x_d[:, t0:t0 + nxc, :])
            return xt

        for t in range(T):
            if t == 0:
                x_cur = load_xchunk(0)
                if T > XCHUNK:
                    x_nxt = load_xchunk(XCHUNK)
            elif t % XCHUNK == 0:
                x_cur = x_nxt
                if t + XCHUNK < T:
                    x_nxt = load_xchunk(t + XCHUNK)
            ti = t % XCHUNK

            # ---- layer 0 step t: per-chunk groups [in, DR kp0, DR kp1]
            # so chunk n's ACT op can start as soon as its 3 matmuls land.
            # The K=33 input matmuls run 2-wide in 64-row PE tiles.
            g0c = []
            for n in range(NS):
                s = slice(n * 512, (n + 1) * 512)
                rb = 64 * (n % 2)
                gcn = pp.tile([128, 512], F32, tag="g")
                mm(gcn[:], x_cur[rb:rb + ENC_IN + 1, ti, :],
                   w0[rb:rb + ENC_IN + 1, s], tile_position=(rb, 0),
                   start=True, stop=(t == 0))
                if t > 0:
                    hp8 = h0T8_hist[t - 1]
                    mm(gcn[:], hp8[:, 0:2, :], wh0[:, 0:2, s],
                       perf_mode=DR, start=False, stop=False)
                    mm(gcn[:], hp8[:, 2:4, :], wh0[:, 2:4, s],
                       perf_mode=DR, start=False, stop=True)
                g0c.append(gcn)
            # L1(t-1) bias mms: K=1, packed 4-wide into 32-row PE tiles.
            g1c = None
            if t >= 1:
                g1c = []
                for n in range(NS):
                    bp = 32 * n
                    gcn = pp.tile([128, 512], F32, tag="g")
                    mm(gcn[:], b14[bp:bp + 1, 0:128], b14[bp:bp + 1, 128:640],
                       tile_position=(bp, 0), start=True, stop=False)
                    g1c.append(gcn)

            def _cast_h1():
                nonlocal h1T8
                if h1T_pending is not None:
                    h1T8 = cast8(h1T_pending, "h1T8")
            h0 = cell(g0c, c0, t == 0, "h0", scale=GSCALE, mid_emit=_cast_h1)

            # ---- layer 1, step t-1 ----
            if t >= 1:
                tp = t - 1
                hp8 = h0T8_hist[tp]
                for n in range(NS):
                    s = slice(n * 512, (n + 1) * 512)
                    gcn = g1c[n]
                    mm(gcn[:], hp8[:, 0:2, :], wi1[:, 0:2, s],
                       perf_mode=DR, start=False, stop=False)
                    mm(gcn[:], hp8[:, 2:4, :], wi1[:, 2:4, s],
                       perf_mode=DR, start=False, stop=(tp == 0))
                    if tp > 0:
                        mm(gcn[:], h1T8[:, 0:2, :], wh1[:, 0:2, s],
                           perf_mode=DR, start=False, stop=False)
                        mm(gcn[:], h1T8[:, 2:4, :], wh1[:, 2:4, s],
                           perf_mode=DR, start=False, stop=True)
            h1T_pending = None
            # PE: transpose h0(t) right after the L1 matmuls (h0 is ready by
            # then); the DVE cast follows cell0's muls in the DVE FIFO.
            h0T_ps = pe_transp(h0, "hTps")
            h0T8_hist[t] = cast8(h0T_ps, "h0T8")
            h0T8_hist.pop(t - 2, None)
            if t >= 1:
                h1 = cell(g1c, c1, tp == 0, "h1", scale=GSCALE)
                # h1's transpose rides the idle DMA xbar (a full iteration
                # of slack) instead of costing PE cycles.
                h1T_pending = htp.tile([128, 4, 128], BF16, tag="h1T")
                nc.sync.dma_start_transpose(h1T_pending[:], h1[:])

        # final L1 step (t = T-1)
        h1T8 = cast8(h1T_pending, "h1T8")
        g1c = []
        for n in range(NS):
            bp = 32 * n
            gcn = pp.tile([128, 512], F32, tag="g")
            mm(gcn[:], b14[bp:bp + 1, 0:128], b14[bp:bp + 1, 128:640],
               tile_position=(bp, 0), start=True, stop=False)
            g1c.append(gcn)
        hp8 = h0T8_hist[T - 1]
        for n in range(NS):
            s = slice(n * 512, (n + 1) * 512)
            gcn = g1c[n]
            mm(gcn[:], hp8[:, 0:2, :], wi1[:, 0:2, s],
               perf_mode=DR, start=False, stop=False)
            mm(gcn[:], hp8[:, 2:4, :], wi1[:, 2:4, s],
               perf_mode=DR, start=False, stop=False)
            mm(gcn[:], h1T8[:, 0:2, :], wh1[:, 0:2, s],
               perf_mode=DR, start=False, stop=False)
            mm(gcn[:], h1T8[:, 2:4, :], wh1[:, 2:4, s],
               perf_mode=DR, start=False, stop=True)
        h1 = cell(g1c, c1, False, "h1", scale=GSCALE)
        h1T = htp.tile([128, 4, 128], BF16, tag="h1T")
        nc.sync.dma_start_transpose(h1T[:], h1[:])

        # ================= decoder (bf16) =================
        wctx = load("wctx", wctx_d, [128, 4, G], BF16)
        whd = load("whd", whd_d, [128, 4, G], BF16)
        covy = load("covy", covy_d, [128, HD, BL], BF16)
        wcy = load("wcy", wcy_d, [128, G], BF16)
        wms = load("wms", wms_d, [128, 2 * HID + 2], F32)
        # one-time: ctx_pre = context @ W_ctx.T + (b_ihd + b_hhd)
        cpsc = []
        for n in range(NS):
            s = slice(n * 512, (n + 1) * 512)
            gcn = pp.tile([128, 512], F32, tag="g")
            mm(gcn[:], ones32_r, bd_r[:, s], start=True, stop=False)
            cpsc.append(gcn)
        for k in range(4):
            for n in range(NS):
                s = slice(n * 512, (n + 1) * 512)
                mm(cpsc[n][:], h1T[:, k, :], wctx[:, k, s],
                   start=False, stop=(k == 3))
        ctxp = cp.tile([128, G], BF16, tag="ctxp")
        for n in range(NS):
            s = slice(n * 512, (n + 1) * 512)
            nc.scalar.copy(ctxp[:, s], cpsc[n][:])

        hdT = None
        for t in range(HD):
            gdc = []
            for n in range(NS):
                s = slice(n * 512, (n + 1) * 512)
                gcn = pp.tile([128, 512], F32, tag="g")
                mm(gcn[:], ident[:], ctxp[:, s], start=True, stop=False)
                rb = 32 * n
                mm(gcn[:], covy[rb:rb + DEC_IN + 1, t, :],
                   wcy[rb:rb + DEC_IN + 1, s], tile_position=(rb, 0),
                   start=False, stop=(t == 0))
                if t > 0:
                    for k in range(4):
                        mm(gcn[:], hdT[:, k, :], whd[:, k, s],
                           start=False, stop=(k == 3))
                gdc.append(gcn)
            hd = cell(gdc, cd, t == 0, "hd")
            hdT_ps = pe_transp(hd, "hTps")
            hdT = cast8(hdT_ps, "hdT", scale=1.0, dt=BF16)

            # heads: mu/sigma dot-products on DVE, off the critical path
            hsc = smp.tile([128, HID], F32, tag="hsc")
            nc.vector.scalar_tensor_tensor(
                hsc[:], hd[:], 1.0, wms[:, 0:HID],
                op0=ALU.mult, op1=ALU.mult, accum_out=mu_b[:, t:t + 1])
            hsc2 = smp.tile([128, HID], F32, tag="hsc2")
            nc.vector.scalar_tensor_tensor(
                hsc2[:], hd[:], 1.0, wms[:, HID:2 * HID],
                op0=ALU.mult, op1=ALU.mult, accum_out=sp_b[:, t:t + 1])

        # add head biases; sigma = softplus(x) + 1e-6 via ln(exp(x)+1)
        nc.vector.tensor_scalar_add(mu_b[:], mu_b[:],
                                    wms[:, 2 * HID:2 * HID + 1])
        nc.vector.tensor_scalar_add(sp_b[:], sp_b[:],
                                    wms[:, 2 * HID + 1:2 * HID + 2])
        nc.scalar.activation(sp_b[:], sp_b[:], AF.Exp)
        nc.scalar.activation(sg_b[:], sp_b[:], AF.Ln, bias=1.0)
        nc.vector.tensor_scalar_add(sg_b[:], sg_b[:], 1e-6)
        nc.sync.dma_start(mu_d[:], mu_b[:])
        nc.sync.dma_start(sg_d[:], sg_b[:])


def _make_be(bdv):
    be = np.zeros((33, G + 128), np.float32)
    be[32, :G] = bdv
    be[32, G:] = 1.0
    return _bf16(be)


def _make_b14(b1):
    """b1*256 chunks at partitions 0/32/64/96 for 4-wide K=1 row tiling."""
    b = np.zeros((128, 640), np.float32)
    for i in range(4):
        b[32 * i, 0:128] = 1.0
        b[32 * i, 128:640] = b1[i * 512:(i + 1) * 512] / GSCALE
    return _bf16(b)


def _make_wms(W_mu, W_sig, b_mu, b_sig):
    w = np.zeros((128, 2 * HID + 2), np.float32)
    w[:, 0:HID] = W_mu[0][None, :]
    w[:, HID:2 * HID] = W_sig[0][None, :]
    w[:, 2 * HID] = b_mu[0]
    w[:, 2 * HID + 1] = b_sig[0]
    return _f32(w)


def prep_inputs(inputs, T=T_ENC, HD=H_DEC):
    """Full-batch inputs -> list of per-core input maps (host layout prep)."""
    enc = _f32(np.asarray(inputs["enc_inp"]))[:, :T]
    dec = _f32(np.asarray(inputs["dec_inp"]))[:, :HD]
    tgt = _f32(np.asarray(inputs["tgt"]))[:, :HD]

    W_ih0, W_hh0 = np.asarray(inputs["W_ih0"]), np.asarray(inputs["W_hh0"])
    W_ih1, W_hh1 = np.asarray(inputs["W_ih1"]), np.asarray(inputs["W_hh1"])
    W_ihd, W_hhd = np.asarray(inputs["W_ihd"]), np.asarray(inputs["W_hhd"])
    b0 = _f32(np.asarray(inputs["b_ih0"]) + np.asarray(inputs["b_hh0"]))[_PERM]
    b1 = _f32(np.asarray(inputs["b_ih1"]) + np.asarray(inputs["b_hh1"]))[_PERM]
    bdv = _f32(np.asarray(inputs["b_ihd"]) + np.asarray(inputs["b_hhd"]))[_PERM]
    W_mu, b_mu = np.asarray(inputs["W_mu"]), np.asarray(inputs["b_mu"])
    W_sig, b_sig = np.asarray(inputs["W_sig"]), np.asarray(inputs["b_sig"])

    # x-side weights *256 in bf16 (exact power-of-two scale); gate reads
    # apply scale=1/256. b0 rides the ones-row of x. w0 replicated at
    # partition 64 for the 2-wide 64-row input matmuls.
    w0 = np.concatenate([W_ih0[_PERM].T, b0[None, :]], 0) / GSCALE  # [33,2048]
    w02 = np.zeros((128, G), np.float32)
    w02[0:ENC_IN + 1] = w0
    w02[64:64 + ENC_IN + 1] = w0
    wcy1 = np.concatenate(
        [W_ihd[_PERM][:, :DEC_IN].T, W_ihd[_PERM][:, DEC_IN + HID:].T], 0)
    wcy4 = np.zeros((128, G), np.float32)
    for i in range(4):
        wcy4[32 * i:32 * i + DEC_IN + 1] = wcy1
    shared = {
        "w0": _bf16(w02),
        "wh0": _wT_kxn(W_hh0, conv=_fp8, scale=WSCALE),
        "wi1": _wT_kxn(W_ih1, conv=_fp8, scale=WSCALE),
        "wh1": _wT_kxn(W_hh1, conv=_fp8, scale=WSCALE),
        "wctx": _wT_kxn(W_ihd[:, DEC_IN:DEC_IN + HID]),
        "whd": _wT_kxn(W_hhd),
        "be": _make_be(bdv),
        "b14": _make_b14(b1),
        "wcy": _bf16(wcy4),
        "wms": _make_wms(W_mu, W_sig, b_mu, b_sig),
    }

    in_maps = []
    for c in range(NCORES):
        sl = slice(c * BL, (c + 1) * BL)
        xe = np.ones((ENC_IN + 1, T, BL), np.float32)
        xe[:ENC_IN] = enc[sl].transpose(2, 1, 0)
        cy1 = np.zeros((DEC_IN + 1, HD, BL), np.float32)
        cy1[:DEC_IN] = dec[sl].transpose(2, 1, 0)
        cy1[DEC_IN, 1:] = tgt[sl, :HD - 1].T
        cy = np.zeros((128, HD, BL), np.float32)
        for i in range(4):
            cy[32 * i:32 * i + DEC_IN + 1] = cy1
        m = dict(shared)
        m["x"] = _bf16(xe)
        m["covy"] = _bf16(cy)
        in_maps.append(m)
    return in_maps


_NC_CACHE = {}


def _get_nc(T=T_ENC, HD=H_DEC):
    key = (T, HD)
    if key not in _NC_CACHE:
        _NC_CACHE[key] = build_kernel(T, HD)
    return _NC_CACHE[key]


def run(inputs, T=T_ENC, HD=H_DEC, **kw):
    nc = _get_nc(T, HD)
    in_maps = prep_inputs(inputs, T, HD)
    res = run_bass_kernel_spmd(nc, in_maps, core_ids=list(range(NCORES)), **kw)
    mu = np.concatenate([res.results[c]["mu"] for c in range(NCORES)], 0)
    sg = np.concatenate([res.results[c]["sg"] for c in range(NCORES)], 0)
    return (mu, sg), res


def kernel(**inputs):
    (mu, sg), _ = run(inputs)
    return mu, sg


# revision 28
# speedup vs baseline: 2.4636x; 1.0005x over previous
"""DeepAR (2-layer LSTM encoder + LSTM-cell decoder) Trainium2 Bass kernel.

Sharding: pure data parallel, batch 1024 -> 128 per core across 8 cores
(batch 128 == SBUF partition width).

Per-core design (v2 — fp8 DoubleRow encoder):
  - gates in [128 batch, 2048 gate] layout, gate order reordered to
    [g, i, f, o]: tanh(g) finishes first so the DVE chain starts early,
    and one sigmoid covers cols 512:2048.
  - encoder recurrent matmuls (h@W_hh0, h0@W_ih1, h1@W_hh1) run in
    fp8e4 with perf_mode=DoubleRow: 2 fp8 weights per PE cell -> ~2x
    matmul throughput. Weights and h both pre-scaled by 16 (so fp8
    mantissa sees normal-range values); the x-side weights/biases are
    scaled by 256 in bf16, and the cell activations apply scale=1/256.
  - h produced in bf16, transposed to stationary [K,M] layout with a
    SINGLE [128,512]->[128,4,128] xbar transpose per h (chunk-major
    layout verified on HW), then cast+scaled to fp8 on the idle Pool
    engine (keeps DVE/ACT FIFOs clean).
  - per-512-col n-chunk accumulation groups emitted so each PSUM bank
    completes early and ACT starts before the whole gate tensor is done.
  - layer 1 runs one step behind layer 0 so the PE always has
    independent matmul work while layer 0's elementwise chain runs.
  - decoder kept in bf16 (24 steps, accuracy headroom): context
    contribution precomputed once and injected into PSUM via identity
    matmul each step; mu/sigma heads are DVE dot-products.
"""
import numpy as np
import ml_dtypes

import concourse.bass as bass
import concourse.mybir as mybir
import concourse.tile as tile
from concourse import bacc
from concourse.bass_utils import run_bass_kernel_spmd
from concourse.masks import make_identity

F32 = mybir.dt.float32
BF16 = mybir.dt.bfloat16
FP8 = mybir.dt.float8e4
AF = mybir.ActivationFunctionType
ALU = mybir.AluOpType
DR = mybir.MatmulPerfMode.DoubleRow

B, T_ENC, H_DEC = 1024, 168, 24
ENC_IN, DEC_IN, HID = 32, 16, 512
G = 4 * HID  # 2048
NCORES = 8
BL = B // NCORES  # 128 batch per core
XCHUNK = 28  # encoder-input steps per DMA chunk

WSCALE = 16.0  # fp8 weight pre-scale
HSCALE = 16.0  # fp8 h pre-scale
GSCALE = 1.0 / (WSCALE * HSCALE)  # ACT de-scale on gate reads

# gate reorder: torch order [i, f, g, o] -> [g, i, f, o]
_PERM = np.concatenate([np.arange(1024, 1536), np.arange(0, 512),
                        np.arange(512, 1024), np.arange(1536, 2048)])


def _bf16(x):
    return np.ascontiguousarray(x.astype(ml_dtypes.bfloat16))


def _fp8(x):
    return np.ascontiguousarray(
        np.clip(x, -224.0, 224.0).astype(ml_dtypes.float8_e4m3))


def _f32(x):
    return np.ascontiguousarray(x.astype(np.float32))


def _wT_kxn(W, conv=_bf16, scale=1.0):
    """[4H, D] gate-major weight -> reordered W.T as [128, D//128, 4H]."""
    Wt = W[_PERM].T * scale  # [D, 2048]
    D = Wt.shape[0]
    return conv(Wt.reshape(D // 128, 128, G).transpose(1, 0, 2))


def build_kernel(T=T_ENC, HD=H_DEC):
    nc = bacc.Bacc("TRN2", target_bir_lowering=False, debug=False,
                   num_devices=NCORES)

    def din(name, shape, dt):
        return nc.dram_tensor(name, shape, dt, kind="ExternalInput").ap()

    x_d = din("x", [ENC_IN + 1, T, BL], BF16)        # enc features + ones row
    w0_d = din("w0", [128, G], BF16)  # (W_ih0T + bias row)*256 @ parts 0,64
    wh0_d = din("wh0", [128, 4, G], FP8)              # *16
    wi1_d = din("wi1", [128, 4, G], FP8)              # *16
    wh1_d = din("wh1", [128, 4, G], FP8)              # *16
    wctx_d = din("wctx", [128, 4, G], BF16)
    whd_d = din("whd", [128, 4, G], BF16)
    be_d = din("be", [33, G + 128], BF16)  # row32: bd|ones (decoder)
    # b1*256 by chunk at partitions 0/32/64/96: cols 0:128 ones, 128:640 bias
    b14_d = din("b14", [128, 640], BF16)
    covy_d = din("covy", [128, HD, BL], BF16)  # dec cov+y at parts 0/32/64/96
    wcy_d = din("wcy", [128, G], BF16)         # replicated at parts 0/32/64/96
    # head weights broadcast across partitions + per-partition biases:
    # cols 0:512 W_mu, 512:1024 W_sig, 1024 b_mu, 1025 b_sig
    wms_d = din("wms", [128, 2 * HID + 2], F32)

    mu_d = nc.dram_tensor("mu", [BL, HD], F32, kind="ExternalOutput").ap()
    sg_d = nc.dram_tensor("sg", [BL, HD], F32, kind="ExternalOutput").ap()

    with tile.TileContext(nc) as tc:
        _emit(tc, T, HD, x_d, w0_d, wh0_d, wi1_d, wh1_d, wctx_d, whd_d,
              be_d, b14_d, covy_d, wcy_d, wms_d, mu_d, sg_d)
    nc.compile()
    return nc


def _emit(tc, T, HD, x_d, w0_d, wh0_d, wi1_d, wh1_d, wctx_d, whd_d,
          be_d, b14_d, covy_d, wcy_d, wms_d, mu_d, sg_d):
    nc = tc.nc
    mm = nc.tensor.matmul

    with (
        tc.tile_pool(name="const", bufs=1) as cp,
        tc.tile_pool(name="xp", bufs=2) as xp,
        tc.tile_pool(name="sig", bufs=3) as sigp,
        tc.tile_pool(name="small", bufs=3) as smp,
        tc.tile_pool(name="hp", bufs=3) as hp,
        tc.tile_pool(name="htp", bufs=3) as htp,
        tc.tile_pool(name="ht8p", bufs=4) as ht8p,
        # gates live as four 1-bank [128,512] chunk tiles: 6 bufs = 6 banks,
        # leaving bank room for the PE-transpose staging tiles below.
        tc.tile_pool(name="psum", bufs=6, space="PSUM") as pp,
        tc.tile_pool(name="htps", bufs=2, space="PSUM") as hTpp,
    ):
        # ---- persistent tiles / weight loads ----
        def load(name, dram, shape, dt):
            t = cp.tile(shape, dt, tag=name)
            nc.sync.dma_start(t[:], dram[:])
            return t

        w0 = load("w0", w0_d, [128, G], BF16)
        wh0 = load("wh0", wh0_d, [128, 4, G], FP8)
        be = load("be", be_d, [33, G + 128], BF16)
        b14 = load("b14", b14_d, [128, 640], BF16)
        wi1 = load("wi1", wi1_d, [128, 4, G], FP8)
        wh1 = load("wh1", wh1_d, [128, 4, G], FP8)

        ident = cp.tile([128, 128], BF16, tag="ident")
        make_identity(nc, ident[:])

        ones32_r = be[32:33, G:G + 128]
        bd_r = be[32:33, 0:G]

        c0 = cp.tile([128, HID], F32, tag="c0")
        c1 = cp.tile([128, HID], F32, tag="c1")
        cd = cp.tile([128, HID], F32, tag="cd")
        mu_b = cp.tile([128, HD], F32, tag="mu_b")
        sp_b = cp.tile([128, HD], F32, tag="sp_b")
        sg_b = cp.tile([128, HD], F32, tag="sg_b")

        NS = G // 512  # 4 n-chunks

        def cell(gc, c, first, h_tag, scale=1.0, mid_emit=None):
            """gate chunk tiles [g, i, f, o] (each [128,512] psum) -> h bf16.

            One ACT op per gate chunk so each starts as soon as its chunk's
            matmuls land; the DVE m1/m2/add chain follows chunk arrivals.
            mid_emit() is called between `add` and the h muls — a natural
            DVE idle gap (DVE waits on ACT tanh(c) there) used to slot in
            the previous h1's fp8 cast.
            """
            tg = smp.tile([128, HID], F32, tag="tg")
            nc.scalar.activation(tg[:], gc[0][:], AF.Tanh, scale=scale)
            si = smp.tile([128, HID], F32, tag="si")
            nc.scalar.activation(si[:], gc[1][:], AF.Sigmoid, scale=scale)
            sf = smp.tile([128, HID], F32, tag="sf")
            nc.scalar.activation(sf[:], gc[2][:], AF.Sigmoid, scale=scale)
            so = sigp.tile([128, HID], F32, tag="so")
            nc.scalar.activation(so[:], gc[3][:], AF.Sigmoid, scale=scale)
            if first:
                nc.vector.tensor_mul(c[:], si[:], tg[:])
            else:
                m1 = smp.tile([128, HID], F32, tag="m1")
                nc.vector.tensor_mul(m1[:], si[:], tg[:])
                m2 = smp.tile([128, HID], F32, tag="m2")
                nc.vector.tensor_mul(m2[:], sf[:], c[:])
                nc.vector.tensor_add(c[:], m1[:], m2[:])
            if mid_emit is not None:
                mid_emit()
            tcn = smp.tile([128, HID], F32, tag="tc")
            nc.scalar.activation(tcn[:], c[:], AF.Tanh)
            h = hp.tile([128, HID], BF16, tag=h_tag)
            hh = HID // 2
            nc.vector.tensor_mul(h[:, 0:hh], so[:, 0:hh], tcn[:, 0:hh])
            nc.vector.tensor_mul(h[:, hh:HID], so[:, hh:HID], tcn[:, hh:HID])
            return h

        def pe_transp(h, tag):
            """h [128,512] bf16 SBUF -> [128,4,128] bf16 PSUM via 4 PE
            transposes (~60ns each) — no DMA queue, no 1.7us xbar latency."""
            ht = hTpp.tile([128, 4, 128], BF16, tag=tag)
            for k in range(4):
                nc.tensor.transpose(ht[:, k, :], h[:, k * 128:(k + 1) * 128],
                                    ident[:])
            return ht

        def cast8(ht_ps, tag, scale=HSCALE, dt=FP8):
            """PSUM hT -> fp8 (x16) SBUF on DVE."""
            ht8 = ht8p.tile([128, 4, 128], dt, tag=tag)
            nc.vector.tensor_scalar_mul(ht8[:], ht_ps[:], scale)
            return ht8

        # ================= encoder =================
        # L1 runs one step behind L0: while L0(t)'s elementwise chain runs
        # on ACT/DVE, the PE stays busy on L1(t-1)'s matmuls.
        h0T8_hist = {}
        h1T8 = None
        h1T_pending = None  # bf16 transposed h1 (SBUF) awaiting fp8 cast

        x_cur = None
        x_nxt = None

        def load_xchunk(t0):
            """x replicated at partitions 0 and 64 for 2-way row tiling."""
            nxc = min(XCHUNK, T - t0)
            xt = xp.tile([128, XCHUNK, BL], BF16, tag="x")
            nc.sync.dma_start(xt[0:ENC_IN + 1, :nxc, :], x_d[:, t0:t0 + nxc, :])
            nc.sync.dma_start(xt[64:64 + ENC_IN + 1, :nxc, :],
                              x_d[:, t0:t0 + nxc, :])
            return xt

        for t in range(T):
            if t == 0:
                x_cur = load_xchunk(0)
                if T > XCHUNK:
                    x_nxt = load_xchunk(XCHUNK)
            elif t % XCHUNK == 0:
                x_cur = x_nxt
                if t + XCHUNK < T:
                    x_nxt = load_xchunk(t + XCHUNK)
            ti = t % XCHUNK

            # ---- layer 0 step t ----
            # All four K=33 input matmuls first (one 64-row-tile PE mode
            # block, 2-wide concurrent), then the DR matmuls chunk-major so
            # chunk n still completes early for its ACT op. Blocking by PE
            # tiling mode avoids per-chunk mode-switch drains.
            g0c = []
            for n in range(NS):
                s = slice(n * 512, (n + 1) * 512)
                rb = 64 * (n % 2)
                gcn = pp.tile([128, 512], F32, tag="g")
                mm(gcn[:], x_cur[rb:rb + ENC_IN + 1, ti, :],
                   w0[rb:rb + ENC_IN + 1, s], tile_position=(rb, 0),
                   start=True, stop=(t == 0))
                g0c.append(gcn)
            if t > 0:
                hp8 = h0T8_hist[t - 1]
                for n in range(NS):
                    s = slice(n * 512, (n + 1) * 512)
                    mm(g0c[n][:], hp8[:, 0:2, :], wh0[:, 0:2, s],
                       perf_mode=DR, start=False, stop=False)
                    mm(g0c[n][:], hp8[:, 2:4, :], wh0[:, 2:4, s],
                       perf_mode=DR, start=False, stop=True)
            # L1(t-1) bias mms: K=1, packed 4-wide into 32-row PE tiles.
            g1c = None
            if t >= 1:
                g1c = []
                for n in range(NS):
                    bp = 32 * n
                    gcn = pp.tile([128, 512], F32, tag="g")
                    mm(gcn[:], b14[bp:bp + 1, 0:128], b14[bp:bp + 1, 128:640],
                       tile_position=(bp, 0), start=True, stop=False)
                    g1c.append(gcn)

            def _cast_h1():
                nonlocal h1T8
                if h1T_pending is not None:
                    h1T8 = cast8(h1T_pending, "h1T8")
            h0 = cell(g0c, c0, t == 0, "h0", scale=GSCALE, mid_emit=_cast_h1)

            # ---- layer 1, step t-1 ----
            if t >= 1:
                tp = t - 1
                hp8 = h0T8_hist[tp]
                for n in range(NS):
                    s = slice(n * 512, (n + 1) * 512)
                    gcn = g1c[n]
                    mm(gcn[:], hp8[:, 0:2, :], wi1[:, 0:2, s],
                       perf_mode=DR, start=False, stop=False)
                    mm(gcn[:], hp8[:, 2:4, :], wi1[:, 2:4, s],
                       perf_mode=DR, start=False, stop=(tp == 0))
                    if tp > 0:
                        mm(gcn[:], h1T8[:, 0:2, :], wh1[:, 0:2, s],
                           perf_mode=DR, start=False, stop=False)
                        mm(gcn[:], h1T8[:, 2:4, :], wh1[:, 2:4, s],
                           perf_mode=DR, start=False, stop=True)
            h1T_pending = None
            # PE: transpose h0(t) right after the L1 matmuls (h0 is ready by
            # then); the DVE cast follows cell0's muls in the DVE FIFO.
            h0T_ps = pe_transp(h0, "hTps")
            h0T8_hist[t] = cast8(h0T_ps, "h0T8")
            h0T8_hist.pop(t - 2, None)
            if t >= 1:
                h1 = cell(g1c, c1, tp == 0, "h1", scale=GSCALE)
                # h1's transpose rides the idle DMA xbar (a full iteration
                # of slack) instead of costing PE cycles.
                h1T_pending = htp.tile([128, 4, 128], BF16, tag="h1T")
                nc.sync.dma_start_transpose(h1T_pending[:], h1[:])

        # final L1 step (t = T-1)
        h1T8 = cast8(h1T_pending, "h1T8")
        g1c = []
        for n in range(NS):
            bp = 32 * n
            gcn = pp.tile([128, 512], F32, tag="g")
            mm(gcn[:], b14[bp:bp + 1, 0:128], b14[bp:bp + 1, 128:640],
               tile_position=(bp, 0), start=True, stop=False)
            g1c.append(gcn)
        hp8 = h0T8_hist[T - 1]
        for n in range(NS):
            s = slice(n * 512, (n + 1) * 512)
            gcn = g1c[n]
            mm(gcn[:], hp8[:, 0:2, :], wi1[:, 0:2, s],
               perf_mode=DR, start=False, stop=False)
            mm(gcn[:], hp8[:, 2:4, :], wi1[:, 2:4, s],
               perf_mode=DR, start=False, stop=False)
            mm(gcn[:], h1T8[:, 0:2, :], wh1[:, 0:2, s],
               perf_mode=DR, start=False, stop=False)
            mm(gcn[:], h1T8[:, 2:4, :], wh1[:, 2:4, s],
               perf_mode=DR, start=False, stop=True)
        h1 = cell(g1c, c1, False, "h1", scale=GSCALE)
        h1T = htp.tile([128, 4, 128], BF16, tag="h1T")
        nc.sync.dma_start_transpose(h1T[:], h1[:])

        # ================= decoder (bf16) =================
        wctx = load("wctx", wctx_d, [128, 4, G], BF16)
        whd = load("whd", whd_d, [128, 4, G], BF16)
        covy = load("covy", covy_d, [128, HD, BL], BF16)
        wcy = load("wcy", wcy_d, [128, G], BF16)
        wms = load("wms", wms_d, [128, 2 * HID + 2], F32)
        # one-time: ctx_pre = context @ W_ctx.T + (b_ihd + b_hhd)
        cpsc = []
        for n in range(NS):
            s = slice(n * 512, (n + 1) * 512)
            gcn = pp.tile([128, 512], F32, tag="g")
            mm(gcn[:], ones32_r, bd_r[:, s], start=True, stop=False)
            cpsc.append(gcn)
        for k in range(4):
            for n in range(NS):
                s = slice(n * 512, (n + 1) * 512)
                mm(cpsc[n][:], h1T[:, k, :], wctx[:, k, s],
                   start=False, stop=(k == 3))
        ctxp = cp.tile([128, G], BF16, tag="ctxp")
        for n in range(NS):
            s = slice(n * 512, (n + 1) * 512)
            nc.scalar.copy(ctxp[:, s], cpsc[n][:])

        hdT = None
        for t in range(HD):
            gdc = []
            for n in range(NS):
                s = slice(n * 512, (n + 1) * 512)
                gcn = pp.tile([128, 512], F32, tag="g")
                mm(gcn[:], ident[:], ctxp[:, s], start=True, stop=False)
                gdc.append(gcn)
            for n in range(NS):
                s = slice(n * 512, (n + 1) * 512)
                rb = 32 * n
                mm(gdc[n][:], covy[rb:rb + DEC_IN + 1, t, :],
                   wcy[rb:rb + DEC_IN + 1, s], tile_position=(rb, 0),
                   start=False, stop=(t == 0))
            if t > 0:
                for n in range(NS):
                    s = slice(n * 512, (n + 1) * 512)
                    for k in range(4):
                        mm(gdc[n][:], hdT[:, k, :], whd[:, k, s],
                           start=False, stop=(k == 3))
            hd = cell(gdc, cd, t == 0, "hd")
            hdT_ps = pe_transp(hd, "hTps")
            hdT = cast8(hdT_ps, "hdT", scale=1.0, dt=BF16)

            # heads: mu/sigma dot-products on DVE, off the critical path
            hsc = smp.tile([128, HID], F32, tag="hsc")
            nc.vector.scalar_tensor_tensor(
                hsc[:], hd[:], 1.0, wms[:, 0:HID],
                op0=ALU.mult, op1=ALU.mult, accum_out=mu_b[:, t:t + 1])
            hsc2 = smp.tile([128, HID], F32, tag="hsc2")
            nc.vector.scalar_tensor_tensor(
                hsc2[:], hd[:], 1.0, wms[:, HID:2 * HID],
                op0=ALU.mult, op1=ALU.mult, accum_out=sp_b[:, t:t + 1])

        # add head biases; sigma = softplus(x) + 1e-6 via ln(exp(x)+1)
        nc.vector.tensor_scalar_add(mu_b[:], mu_b[:],
                                    wms[:, 2 * HID:2 * HID + 1])
        nc.vector.tensor_scalar_add(sp_b[:], sp_b[:],
                                    wms[:, 2 * HID + 1:2 * HID + 2])
        nc.scalar.activation(sp_b[:], sp_b[:], AF.Exp)
        nc.scalar.activation(sg_b[:], sp_b[:], AF.Ln, bias=1.0)
        nc.vector.tensor_scalar_add(sg_b[:], sg_b[:], 1e-6)
        nc.sync.dma_start(mu_d[:], mu_b[:])
        nc.sync.dma_start(sg_d[:], sg_b[:])


def _make_be(bdv):
    be = np.zeros((33, G + 128), np.float32)
    be[32, :G] = bdv
    be[32, G:] = 1.0
    return _bf16(be)


def _make_b14(b1):
    """b1*256 chunks at partitions 0/32/64/96 for 4-wide K=1 row tiling."""
    b = np.zeros((128, 640), np.float32)
    for i in range(4):
        b[32 * i, 0:128] = 1.0
        b[32 * i, 128:640] = b1[i * 512:(i + 1) * 512] / GSCALE
    return _bf16(b)


def _make_wms(W_mu, W_sig, b_mu, b_sig):
    w = np.zeros((128, 2 * HID + 2), np.float32)
    w[:, 0:HID] = W_mu[0][None, :]
    w[:, HID:2 * HID] = W_sig[0][None, :]
    w[:, 2 * HID] = b_mu[0]
    w[:, 2 * HID + 1] = b_sig[0]
    return _f32(w)


def prep_inputs(inputs, T=T_ENC, HD=H_DEC):
    """Full-batch inputs -> list of per-core input maps (host layout prep)."""
    enc = _f32(np.asarray(inputs["enc_inp"]))[:, :T]
    dec = _f32(np.asarray(inputs["dec_inp"]))[:, :HD]
    tgt = _f32(np.asarray(inputs["tgt"]))[:, :HD]

    W_ih0, W_hh0 = np.asarray(inputs["W_ih0"]), np.asarray(inputs["W_hh0"])
    W_ih1, W_hh1 = np.asarray(inputs["W_ih1"]), np.asarray(inputs["W_hh1"])
    W_ihd, W_hhd = np.asarray(inputs["W_ihd"]), np.asarray(inputs["W_hhd"])
    b0 = _f32(np.asarray(inputs["b_ih0"]) + np.asarray(inputs["b_hh0"]))[_PERM]
    b1 = _f32(np.asarray(inputs["b_ih1"]) + np.asarray(inputs["b_hh1"]))[_PERM]
    bdv = _f32(np.asarray(inputs["b_ihd"]) + np.asarray(inputs["b_hhd"]))[_PERM]
    W_mu, b_mu = np.asarray(inputs["W_mu"]), np.asarray(inputs["b_mu"])
    W_sig, b_sig = np.asarray(inputs["W_sig"]), np.asarray(inputs["b_sig"])

    # x-side weights *256 in bf16 (exact power-of-two scale); gate reads
    # apply scale=1/256. b0 rides the ones-row of x. w0 replicated at
    # partition 64 for the 2-wide 64-row input matmuls.
    w0 = np.concatenate([W_ih0[_PERM].T, b0[None, :]], 0) / GSCALE  # [33,2048]
    w02 = np.zeros((128, G), np.float32)
    w02[0:ENC_IN + 1] = w0
    w02[64:64 + ENC_IN + 1] = w0
    wcy1 = np.concatenate(
        [W_ihd[_PERM][:, :DEC_IN].T, W_ihd[_PERM][:, DEC_IN + HID:].T], 0)
    wcy4 = np.zeros((128, G), np.float32)
    for i in range(4):
        wcy4[32 * i:32 * i + DEC_IN + 1] = wcy1
    shared = {
        "w0": _bf16(w02),
        "wh0": _wT_kxn(W_hh0, conv=_fp8, scale=WSCALE),
        "wi1": _wT_kxn(W_ih1, conv=_fp8, scale=WSCALE),
        "wh1": _wT_kxn(W_hh1, conv=_fp8, scale=WSCALE),
        "wctx": _wT_kxn(W_ihd[:, DEC_IN:DEC_IN + HID]),
        "whd": _wT_kxn(W_hhd),
        "be": _make_be(bdv),
        "b14": _make_b14(b1),
        "wcy": _bf16(wcy4),
        "wms": _make_wms(W_mu, W_sig, b_mu, b_sig),
    }

    in_maps = []
    for c in range(NCORES):
        sl = slice(c * BL, (c + 1) * BL)
        xe = np.ones((ENC_IN + 1, T, BL), np.float32)
        xe[:ENC_IN] = enc[sl].transpose(2, 1, 0)
        cy1 = np.zeros((DEC_IN + 1, HD, BL), np.float32)
        cy1[:DEC_IN] = dec[sl].transpose(2, 1, 0)
        cy1[DEC_IN, 1:] = tgt[sl, :HD - 1].T
        cy = np.zeros((128, HD, BL), np.float32)
        for i in range(4):
            cy[32 * i:32 * i + DEC_IN + 1] = cy1
        m = dict(shared)
        m["x"] = _bf16(xe)
        m["covy"] = _bf16(cy)
        in_maps.append(m)
    return in_maps


_NC_CACHE = {}


def _get_nc(T=T_ENC, HD=H_DEC):
    key = (T, HD)
    if key not in _NC_CACHE:
        _NC_CACHE[key] = build_kernel(T, HD)
    return _NC_CACHE[key]


def run(inputs, T=T_ENC, HD=H_DEC, **kw):
    nc = _get_nc(T, HD)
    in_maps = prep_inputs(inputs, T, HD)
    res = run_bass_kernel_spmd(nc, in_maps, core_ids=list(range(NCORES)), **kw)
    mu = np.concatenate([res.results[c]["mu"] for c in range(NCORES)], 0)
    sg = np.concatenate([res.results[c]["sg"] for c in range(NCORES)], 0)
    return (mu, sg), res


def kernel(**inputs):
    (mu, sg), _ = run(inputs)
    return mu, sg


# revision 33
# speedup vs baseline: 2.4939x; 1.0123x over previous
"""DeepAR (2-layer LSTM encoder + LSTM-cell decoder) Trainium2 Bass kernel.

Sharding: pure data parallel, batch 1024 -> 128 per core across 8 cores
(batch 128 == SBUF partition width).

Per-core design (fp8-DoubleRow encoder, chain-tightened):
  - gates in [128 batch, 2048 gate] layout, reordered to [g, i, f, o];
    each 512-col gate chunk is its own 1-bank PSUM tile so its ACT op
    starts as soon as that chunk's matmuls land.
  - encoder recurrent matmuls (h@W_hh0, h0@W_ih1, h1@W_hh1) run in
    fp8e4 with perf_mode=DoubleRow (~1.6x measured over bf16). Weights
    and h pre-scaled by 16 each (fp8 mantissa in normal range); x-side
    weights/biases scaled by 256 in bf16; gate ACT ops apply 1/256.
  - h0 (chain-critical) is transposed by 4 PE transposes into a PSUM
    staging tile, then DVE-cast to fp8 SBUF (x16) — no 1.7us DMA-xbar
    latency on the recurrence cycle. h1 (one iteration of slack) uses
    a single [128,512]->[128,4,128] chunk-major xbar transpose on the
    otherwise-idle sync DMA queue, cast on DVE in the add->muls gap.
  - K=1 bias matmuls packed 4-wide into 32-row PE tiles; K=33 encoder
    input matmuls 2-wide into 64-row tiles (x/w0 replicated at the
    matching partition offsets); same-mode matmuls emitted in blocks.
  - layer 1 runs one step behind layer 0 so the PE always has
    independent matmul work while layer 0's elementwise chain runs.
  - decoder kept in bf16 (24 steps, accuracy headroom): context
    contribution precomputed once and injected via identity matmul;
    K=17 covariate matmuls packed 4-wide; hd PE-transposed; mu/sigma
    heads are DVE dot-products with accumulate.
  - avoid GPSIMD for fp8 work: its software tensor ops are ~7.5us per
    512-el instruction and it throttles the whole pipeline.
"""
import numpy as np
import ml_dtypes

import concourse.bass as bass
import concourse.mybir as mybir
import concourse.tile as tile
from concourse import bacc
from concourse.bass_utils import run_bass_kernel_spmd
from concourse.masks import make_identity

F32 = mybir.dt.float32
BF16 = mybir.dt.bfloat16
FP8 = mybir.dt.float8e4
AF = mybir.ActivationFunctionType
ALU = mybir.AluOpType
DR = mybir.MatmulPerfMode.DoubleRow

B, T_ENC, H_DEC = 1024, 168, 24
ENC_IN, DEC_IN, HID = 32, 16, 512
G = 4 * HID  # 2048
NCORES = 8
BL = B // NCORES  # 128 batch per core
XCHUNK = 28  # encoder-input steps per DMA chunk

WSCALE = 16.0  # fp8 weight pre-scale
HSCALE = 16.0  # fp8 h pre-scale
GSCALE = 1.0 / (WSCALE * HSCALE)  # ACT de-scale on gate reads

# gate reorder: torch order [i, f, g, o] -> [g, i, f, o]
_PERM = np.concatenate([np.arange(1024, 1536), np.arange(0, 512),
                        np.arange(512, 1024), np.arange(1536, 2048)])


def _bf16(x):
    return np.ascontiguousarray(x.astype(ml_dtypes.bfloat16))


def _fp8(x):
    return np.ascontiguousarray(
        np.clip(x, -224.0, 224.0).astype(ml_dtypes.float8_e4m3))


def _f32(x):
    return np.ascontiguousarray(x.astype(np.float32))


def _wT_kxn(W, conv=_bf16, scale=1.0):
    """[4H, D] gate-major weight -> reordered W.T as [128, D//128, 4H]."""
    Wt = W[_PERM].T * scale  # [D, 2048]
    D = Wt.shape[0]
    return conv(Wt.reshape(D // 128, 128, G).transpose(1, 0, 2))


def build_kernel(T=T_ENC, HD=H_DEC):
    nc = bacc.Bacc("TRN2", target_bir_lowering=False, debug=False,
                   num_devices=NCORES)

    def din(name, shape, dt):
        return nc.dram_tensor(name, shape, dt, kind="ExternalInput").ap()

    x_d = din("x", [ENC_IN + 1, T, BL], BF16)        # enc features + ones row
    w0_d = din("w0", [128, G], BF16)  # (W_ih0T + bias row)*256 @ parts 0,64
    wh0_d = din("wh0", [128, 4, G], FP8)              # *16
    wi1_d = din("wi1", [128, 4, G], FP8)              # *16
    wh1_d = din("wh1", [128, 4, G], FP8)              # *16
    wctx_d = din("wctx", [128, 4, G], BF16)
    whd_d = din("whd", [128, 4, G], BF16)
    be_d = din("be", [33, G + 128], BF16)  # row32: bd|ones (decoder)
    # b1*256 by chunk at partitions 0/32/64/96: cols 0:128 ones, 128:640 bias
    b14_d = din("b14", [128, 640], BF16)
    covy_d = din("covy", [128, HD, BL], BF16)  # dec cov+y at parts 0/32/64/96
    wcy_d = din("wcy", [128, G], BF16)         # replicated at parts 0/32/64/96
    # head weights broadcast across partitions + per-partition biases:
    # cols 0:512 W_mu, 512:1024 W_sig, 1024 b_mu, 1025 b_sig
    wms_d = din("wms", [128, 2 * HID + 2], F32)

    mu_d = nc.dram_tensor("mu", [BL, HD], F32, kind="ExternalOutput").ap()
    sg_d = nc.dram_tensor("sg", [BL, HD], F32, kind="ExternalOutput").ap()

    with tile.TileContext(nc) as tc:
        _emit(tc, T, HD, x_d, w0_d, wh0_d, wi1_d, wh1_d, wctx_d, whd_d,
              be_d, b14_d, covy_d, wcy_d, wms_d, mu_d, sg_d)
    nc.compile()
    return nc


def _emit(tc, T, HD, x_d, w0_d, wh0_d, wi1_d, wh1_d, wctx_d, whd_d,
          be_d, b14_d, covy_d, wcy_d, wms_d, mu_d, sg_d):
    nc = tc.nc
    mm = nc.tensor.matmul

    with (
        tc.tile_pool(name="const", bufs=1) as cp,
        tc.tile_pool(name="xp", bufs=2) as xp,
        tc.tile_pool(name="sig", bufs=3) as sigp,
        tc.tile_pool(name="small", bufs=3) as smp,
        tc.tile_pool(name="hp", bufs=3) as hp,
        tc.tile_pool(name="htp", bufs=3) as htp,
        tc.tile_pool(name="ht8p", bufs=4) as ht8p,
        # gates live as four 1-bank [128,512] chunk tiles: 7 bufs = 7 banks
        # + 1 bank for the single h0/hd PE-transpose staging tile.
        tc.tile_pool(name="psum", bufs=7, space="PSUM") as pp,
        tc.tile_pool(name="htps", bufs=1, space="PSUM") as hTpp,
    ):
        # ---- persistent tiles / weight loads ----
        def load(name, dram, shape, dt):
            t = cp.tile(shape, dt, tag=name)
            nc.sync.dma_start(t[:], dram[:])
            return t

        w0 = load("w0", w0_d, [128, G], BF16)
        wh0 = load("wh0", wh0_d, [128, 4, G], FP8)
        be = load("be", be_d, [33, G + 128], BF16)
        b14 = load("b14", b14_d, [128, 640], BF16)
        wi1 = load("wi1", wi1_d, [128, 4, G], FP8)
        wh1 = load("wh1", wh1_d, [128, 4, G], FP8)

        ident = cp.tile([128, 128], BF16, tag="ident")
        make_identity(nc, ident[:])

        ones32_r = be[32:33, G:G + 128]
        bd_r = be[32:33, 0:G]

        c0 = cp.tile([128, HID], F32, tag="c0")
        c1 = cp.tile([128, HID], F32, tag="c1")
        cd = cp.tile([128, HID], F32, tag="cd")
        mu_b = cp.tile([128, HD], F32, tag="mu_b")
        sp_b = cp.tile([128, HD], F32, tag="sp_b")
        sg_b = cp.tile([128, HD], F32, tag="sg_b")

        NS = G // 512  # 4 n-chunks

        def cell(gc, c, first, h_tag, scale=1.0, mid_emit=None):
            """gate chunk tiles [g, i, f, o] (each [128,512] psum) -> h bf16.

            One ACT op per gate chunk so each starts as soon as its chunk's
            matmuls land; the DVE m1/m2/add chain follows chunk arrivals.
            mid_emit() is called between `add` and the h muls — a natural
            DVE idle gap (DVE waits on ACT tanh(c) there) used to slot in
            the previous h1's fp8 cast.
            """
            tg = smp.tile([128, HID], F32, tag="tg")
            nc.scalar.activation(tg[:], gc[0][:], AF.Tanh, scale=scale)
            si = smp.tile([128, HID], F32, tag="si")
            nc.scalar.activation(si[:], gc[1][:], AF.Sigmoid, scale=scale)
            sf = smp.tile([128, HID], F32, tag="sf")
            nc.scalar.activation(sf[:], gc[2][:], AF.Sigmoid, scale=scale)
            so = sigp.tile([128, HID], F32, tag="so")
            nc.scalar.activation(so[:], gc[3][:], AF.Sigmoid, scale=scale)
            if first:
                nc.vector.tensor_mul(c[:], si[:], tg[:])
            else:
                m1 = smp.tile([128, HID], F32, tag="m1")
                nc.vector.tensor_mul(m1[:], si[:], tg[:])
                m2 = smp.tile([128, HID], F32, tag="m2")
                nc.vector.tensor_mul(m2[:], sf[:], c[:])
                nc.vector.tensor_add(c[:], m1[:], m2[:])
            if mid_emit is not None:
                mid_emit()
            tcn = smp.tile([128, HID], F32, tag="tc")
            nc.scalar.activation(tcn[:], c[:], AF.Tanh)
            h = hp.tile([128, HID], BF16, tag=h_tag)
            hh = HID // 2
            nc.vector.tensor_mul(h[:, 0:hh], so[:, 0:hh], tcn[:, 0:hh])
            nc.vector.tensor_mul(h[:, hh:HID], so[:, hh:HID], tcn[:, hh:HID])
            return h

        def pe_transp(h, tag):
            """h [128,512] bf16 SBUF -> [128,4,128] bf16 PSUM via 4 PE
            transposes (~60ns each) — no DMA queue, no 1.7us xbar latency."""
            ht = hTpp.tile([128, 4, 128], BF16, tag=tag)
            for k in range(4):
                nc.tensor.transpose(ht[:, k, :], h[:, k * 128:(k + 1) * 128],
                                    ident[:])
            return ht

        def cast8(ht_ps, tag, scale=HSCALE, dt=FP8):
            """PSUM hT -> fp8 (x16) SBUF on DVE."""
            ht8 = ht8p.tile([128, 4, 128], dt, tag=tag)
            nc.vector.tensor_scalar_mul(ht8[:], ht_ps[:], scale)
            return ht8

        # ================= encoder =================
        # L1 runs one step behind L0: while L0(t)'s elementwise chain runs
        # on ACT/DVE, the PE stays busy on L1(t-1)'s matmuls.
        h0T8_hist = {}
        h1T8 = None
        h1T_pending = None  # bf16 transposed h1 (SBUF) awaiting fp8 cast

        x_cur = None
        x_nxt = None

        def load_xchunk(t0):
            """x replicated at partitions 0 and 64 for 2-way row tiling."""
            nxc = min(XCHUNK, T - t0)
            xt = xp.tile([128, XCHUNK, BL], BF16, tag="x")
            nc.sync.dma_start(xt[0:ENC_IN + 1, :nxc, :], x_d[:, t0:t0 + nxc, :])
            nc.sync.dma_start(xt[64:64 + ENC_IN + 1, :nxc, :],
                              x_d[:, t0:t0 + nxc, :])
            return xt

        for t in range(T):
            if t == 0:
                x_cur = load_xchunk(0)
                if T > XCHUNK:
                    x_nxt = load_xchunk(XCHUNK)
            elif t % XCHUNK == 0:
                x_cur = x_nxt
                if t + XCHUNK < T:
                    x_nxt = load_xchunk(t + XCHUNK)
            ti = t % XCHUNK

            # ---- layer 0 step t ----
            # All four K=33 input matmuls first (one 64-row-tile PE mode
            # block, 2-wide concurrent), then the DR matmuls chunk-major so
            # chunk n still completes early for its ACT op. Blocking by PE
            # tiling mode avoids per-chunk mode-switch drains.
            g0c = []
            for n in range(NS):
                s = slice(n * 512, (n + 1) * 512)
                rb = 64 * (n % 2)
                gcn = pp.tile([128, 512], F32, tag="g")
                mm(gcn[:], x_cur[rb:rb + ENC_IN + 1, ti, :],
                   w0[rb:rb + ENC_IN + 1, s], tile_position=(rb, 0),
                   start=True, stop=(t == 0))
                g0c.append(gcn)
            # L1(t-1) bias mms: K=1, packed 4-wide into 32-row PE tiles.
            # Emitted before the L0 DR matmuls so the PE has filler work
            # while the h0T8 cast from last iteration lands.
            g1c = None
            if t >= 1:
                g1c = []
                for n in range(NS):
                    bp = 32 * n
                    gcn = pp.tile([128, 512], F32, tag="g")
                    mm(gcn[:], b14[bp:bp + 1, 0:128], b14[bp:bp + 1, 128:640],
                       tile_position=(bp, 0), start=True, stop=False)
                    g1c.append(gcn)
            # h1T8 cast as the iteration's first DVE op: DVE is idle here
            # (cell0's m1 waits on ACT), and the L1-rec matmuls get their
            # stationary ~4us earlier than a mid-cell cast would deliver.
            if h1T_pending is not None:
                h1T8 = cast8(h1T_pending, "h1T8")
                h1T_pending = None
            if t > 0:
                hp8 = h0T8_hist[t - 1]
                for n in range(NS):
                    s = slice(n * 512, (n + 1) * 512)
                    mm(g0c[n][:], hp8[:, 0:2, :], wh0[:, 0:2, s],
                       perf_mode=DR, start=False, stop=False)
                    mm(g0c[n][:], hp8[:, 2:4, :], wh0[:, 2:4, s],
                       perf_mode=DR, start=False, stop=True)
            h0 = cell(g0c, c0, t == 0, "h0", scale=GSCALE)

            # ---- layer 1, step t-1 ----
            if t >= 1:
                tp = t - 1
                hp8 = h0T8_hist[tp]
                for n in range(NS):
                    s = slice(n * 512, (n + 1) * 512)
                    gcn = g1c[n]
                    mm(gcn[:], hp8[:, 0:2, :], wi1[:, 0:2, s],
                       perf_mode=DR, start=False, stop=False)
                    mm(gcn[:], hp8[:, 2:4, :], wi1[:, 2:4, s],
                       perf_mode=DR, start=False, stop=(tp == 0))
                    if tp > 0:
                        mm(gcn[:], h1T8[:, 0:2, :], wh1[:, 0:2, s],
                           perf_mode=DR, start=False, stop=False)
                        mm(gcn[:], h1T8[:, 2:4, :], wh1[:, 2:4, s],
                           perf_mode=DR, start=False, stop=True)
            # PE: transpose h0(t) right after the L1 matmuls (h0 is ready by
            # then); the DVE cast follows cell0's muls in the DVE FIFO.
            h0T_ps = pe_transp(h0, "hTps")
            h0T8_hist[t] = cast8(h0T_ps, "h0T8")
            h0T8_hist.pop(t - 2, None)
            if t >= 1:
                h1 = cell(g1c, c1, tp == 0, "h1", scale=GSCALE)
                # h1's transpose rides the idle DMA xbar (a full iteration
                # of slack) instead of costing PE cycles.
                h1T_pending = htp.tile([128, 4, 128], BF16, tag="h1T")
                nc.sync.dma_start_transpose(h1T_pending[:], h1[:])

        # final L1 step (t = T-1)
        h1T8 = cast8(h1T_pending, "h1T8")
        g1c = []
        for n in range(NS):
            bp = 32 * n
            gcn = pp.tile([128, 512], F32, tag="g")
            mm(gcn[:], b14[bp:bp + 1, 0:128], b14[bp:bp + 1, 128:640],
               tile_position=(bp, 0), start=True, stop=False)
            g1c.append(gcn)
        hp8 = h0T8_hist[T - 1]
        for n in range(NS):
            s = slice(n * 512, (n + 1) * 512)
            gcn = g1c[n]
            mm(gcn[:], hp8[:, 0:2, :], wi1[:, 0:2, s],
               perf_mode=DR, start=False, stop=False)
            mm(gcn[:], hp8[:, 2:4, :], wi1[:, 2:4, s],
               perf_mode=DR, start=False, stop=False)
            mm(gcn[:], h1T8[:, 0:2, :], wh1[:, 0:2, s],
               perf_mode=DR, start=False, stop=False)
            mm(gcn[:], h1T8[:, 2:4, :], wh1[:, 2:4, s],
               perf_mode=DR, start=False, stop=True)
        h1 = cell(g1c, c1, False, "h1", scale=GSCALE)
        h1T = htp.tile([128, 4, 128], BF16, tag="h1T")
        nc.sync.dma_start_transpose(h1T[:], h1[:])

        # ================= decoder (bf16) =================
        wctx = load("wctx", wctx_d, [128, 4, G], BF16)
        whd = load("whd", whd_d, [128, 4, G], BF16)
        covy = load("covy", covy_d, [128, HD, BL], BF16)
        wcy = load("wcy", wcy_d, [128, G], BF16)
        wms = load("wms", wms_d, [128, 2 * HID + 2], F32)
        # one-time: ctx_pre = context @ W_ctx.T + (b_ihd + b_hhd)
        cpsc = []
        for n in range(NS):
            s = slice(n * 512, (n + 1) * 512)
            gcn = pp.tile([128, 512], F32, tag="g")
            mm(gcn[:], ones32_r, bd_r[:, s], start=True, stop=False)
            cpsc.append(gcn)
        for k in range(4):
            for n in range(NS):
                s = slice(n * 512, (n + 1) * 512)
                mm(cpsc[n][:], h1T[:, k, :], wctx[:, k, s],
                   start=False, stop=(k == 3))
        ctxp = cp.tile([128, G], BF16, tag="ctxp")
        for n in range(NS):
            s = slice(n * 512, (n + 1) * 512)
            nc.scalar.copy(ctxp[:, s], cpsc[n][:])

        hdT = None
        for t in range(HD):
            gdc = []
            for n in range(NS):
                s = slice(n * 512, (n + 1) * 512)
                gcn = pp.tile([128, 512], F32, tag="g")
                mm(gcn[:], ident[:], ctxp[:, s], start=True, stop=False)
                gdc.append(gcn)
            for n in range(NS):
                s = slice(n * 512, (n + 1) * 512)
                rb = 32 * n
                mm(gdc[n][:], covy[rb:rb + DEC_IN + 1, t, :],
                   wcy[rb:rb + DEC_IN + 1, s], tile_position=(rb, 0),
                   start=False, stop=(t == 0))
            if t > 0:
                for n in range(NS):
                    s = slice(n * 512, (n + 1) * 512)
                    for k in range(4):
                        mm(gdc[n][:], hdT[:, k, :], whd[:, k, s],
                           start=False, stop=(k == 3))
            hd = cell(gdc, cd, t == 0, "hd")
            hdT_ps = pe_transp(hd, "hTps")
            hdT = cast8(hdT_ps, "hdT", scale=1.0, dt=BF16)

            # heads: mu/sigma dot-products on DVE, off the critical path
            hsc = smp.tile([128, HID], F32, tag="hsc")
            nc.vector.scalar_tensor_tensor(
                hsc[:], hd[:], 1.0, wms[:, 0:HID],
                op0=ALU.mult, op1=ALU.mult, accum_out=mu_b[:, t:t + 1])
            hsc2 = smp.tile([128, HID], F32, tag="hsc2")
            nc.vector.scalar_tensor_tensor(
                hsc2[:], hd[:], 1.0, wms[:, HID:2 * HID],
                op0=ALU.mult, op1=ALU.mult, accum_out=sp_b[:, t:t + 1])

        # add head biases; sigma = softplus(x) + 1e-6 via ln(exp(x)+1)
        nc.vector.tensor_scalar_add(mu_b[:], mu_b[:],
                                    wms[:, 2 * HID:2 * HID + 1])
        nc.vector.tensor_scalar_add(sp_b[:], sp_b[:],
                                    wms[:, 2 * HID + 1:2 * HID + 2])
        nc.scalar.activation(sp_b[:], sp_b[:], AF.Exp)
        nc.scalar.activation(sg_b[:], sp_b[:], AF.Ln, bias=1.0)
        nc.vector.tensor_scalar_add(sg_b[:], sg_b[:], 1e-6)
        nc.sync.dma_start(mu_d[:], mu_b[:])
        nc.sync.dma_start(sg_d[:], sg_b[:])


def _make_be(bdv):
    be = np.zeros((33, G + 128), np.float32)
    be[32, :G] = bdv
    be[32, G:] = 1.0
    return _bf16(be)


def _make_b14(b1):
    """b1*256 chunks at partitions 0/32/64/96 for 4-wide K=1 row tiling."""
    b = np.zeros((128, 640), np.float32)
    for i in range(4):
        b[32 * i, 0:128] = 1.0
        b[32 * i, 128:640] = b1[i * 512:(i + 1) * 512] / GSCALE
    return _bf16(b)


def _make_wms(W_mu, W_sig, b_mu, b_sig):
    w = np.zeros((128, 2 * HID + 2), np.float32)
    w[:, 0:HID] = W_mu[0][None, :]
    w[:, HID:2 * HID] = W_sig[0][None, :]
    w[:, 2 * HID] = b_mu[0]
    w[:, 2 * HID + 1] = b_sig[0]
    return _f32(w)


def prep_inputs(inputs, T=T_ENC, HD=H_DEC):
    """Full-batch inputs -> list of per-core input maps (host layout prep)."""
    enc = _f32(np.asarray(inputs["enc_inp"]))[:, :T]
    dec = _f32(np.asarray(inputs["dec_inp"]))[:, :HD]
    tgt = _f32(np.asarray(inputs["tgt"]))[:, :HD]

    W_ih0, W_hh0 = np.asarray(inputs["W_ih0"]), np.asarray(inputs["W_hh0"])
    W_ih1, W_hh1 = np.asarray(inputs["W_ih1"]), np.asarray(inputs["W_hh1"])
    W_ihd, W_hhd = np.asarray(inputs["W_ihd"]), np.asarray(inputs["W_hhd"])
    b0 = _f32(np.asarray(inputs["b_ih0"]) + np.asarray(inputs["b_hh0"]))[_PERM]
    b1 = _f32(np.asarray(inputs["b_ih1"]) + np.asarray(inputs["b_hh1"]))[_PERM]
    bdv = _f32(np.asarray(inputs["b_ihd"]) + np.asarray(inputs["b_hhd"]))[_PERM]
    W_mu, b_mu = np.asarray(inputs["W_mu"]), np.asarray(inputs["b_mu"])
    W_sig, b_sig = np.asarray(inputs["W_sig"]), np.asarray(inputs["b_sig"])

    # x-side weights *256 in bf16 (exact power-of-two scale); gate reads
    # apply scale=1/256. b0 rides the ones-row of x. w0 replicated at
    # partition 64 for the 2-wide 64-row input matmuls.
    w0 = np.concatenate([W_ih0[_PERM].T, b0[None, :]], 0) / GSCALE  # [33,2048]
    w02 = np.zeros((128, G), np.float32)
    w02[0:ENC_IN + 1] = w0
    w02[64:64 + ENC_IN + 1] = w0
    wcy1 = np.concatenate(
        [W_ihd[_PERM][:, :DEC_IN].T, W_ihd[_PERM][:, DEC_IN + HID:].T], 0)
    wcy4 = np.zeros((128, G), np.float32)
    for i in range(4):
        wcy4[32 * i:32 * i + DEC_IN + 1] = wcy1
    shared = {
        "w0": _bf16(w02),
        "wh0": _wT_kxn(W_hh0, conv=_fp8, scale=WSCALE),
        "wi1": _wT_kxn(W_ih1, conv=_fp8, scale=WSCALE),
        "wh1": _wT_kxn(W_hh1, conv=_fp8, scale=WSCALE),
        "wctx": _wT_kxn(W_ihd[:, DEC_IN:DEC_IN + HID]),
        "whd": _wT_kxn(W_hhd),
        "be": _make_be(bdv),
        "b14": _make_b14(b1),
        "wcy": _bf16(wcy4),
        "wms": _make_wms(W_mu, W_sig, b_mu, b_sig),
    }

    in_maps = []
    for c in range(NCORES):
        sl = slice(c * BL, (c + 1) * BL)
        xe = np.ones((ENC_IN + 1, T, BL), np.float32)
        xe[:ENC_IN] = enc[sl].transpose(2, 1, 0)
        cy1 = np.zeros((DEC_IN + 1, HD, BL), np.float32)
        cy1[:DEC_IN] = dec[sl].transpose(2, 1, 0)
        cy1[DEC_IN, 1:] = tgt[sl, :HD - 1].T
        cy = np.zeros((128, HD, BL), np.float32)
        for i in range(4):
            cy[32 * i:32 * i + DEC_IN + 1] = cy1
        m = dict(shared)
        m["x"] = _bf16(xe)
        m["covy"] = _bf16(cy)
        in_maps.append(m)
    return in_maps


_NC_CACHE = {}


def _get_nc(T=T_ENC, HD=H_DEC):
    key = (T, HD)
    if key not in _NC_CACHE:
        _NC_CACHE[key] = build_kernel(T, HD)
    return _NC_CACHE[key]


def run(inputs, T=T_ENC, HD=H_DEC, **kw):
    nc = _get_nc(T, HD)
    in_maps = prep_inputs(inputs, T, HD)
    res = run_bass_kernel_spmd(nc, in_maps, core_ids=list(range(NCORES)), **kw)
    mu = np.concatenate([res.results[c]["mu"] for c in range(NCORES)], 0)
    sg = np.concatenate([res.results[c]["sg"] for c in range(NCORES)], 0)
    return (mu, sg), res


def kernel(**inputs):
    (mu, sg), _ = run(inputs)
    return mu, sg
